# revision 4
# baseline (speedup 1.0000x reference)
"""Bass/Trainium2 kernel for a BiLSTM (TF-LSTMCell) cross-entropy loss.

Model (per reference):
  x = emb[inputs]                        # [B,T,E]
  h_fw = LSTM(x, Wk_f, b_f)              # forward over T
  h_bw = reverse(LSTM(reverse(x), Wk_b, b_b))
  logits = concat(h_fw, h_bw) @ W + b    # [B,T,2]
  loss = mean(xent(logits, outputs) * mask)

Sharding: data-parallel over batch. B=64 -> 8 cores x 8 rows.  Each core runs
both LSTM directions (two independent chains interleaved to hide latency) and
produces the pre-bias logits for its tokens; the host adds b, computes the
log-softmax cross entropy in float64 and averages (same split as summing the 8
per-core partials: the device does all O(B*T*H) work, the host the O(B*T) tail,
which also avoids an ACT-table swap for LN on device).

Device layout is feature-major: gate/feature index on the SBUF partition axis,
(time x batch) on the free axis, so per-step elementwise work is [128, small]
and the recurrent matmul keeps the weight stationary (bf16 -> fast weight
load).  z pre-activations accumulate in PSUM in 8-step blocks:
  psum col = m*64 + l*8 + b   (l=step-in-block, m=gate chunk of 128, b=batch)
Gate order is host-permuted to [o, i, f, j] so sigmoid covers one contiguous
[128,48] slice and tanh(j) one [128,16].  Weight prep (forget bias +1, j-gate
x2 for the tanh-via-sigmoid trick, bf16 cast, permutation) happens on the host
so weights are DMA-ready.  x-projection and bias are injected into each PSUM
block ahead of time by large-N matmuls (bias via a K=1 ones-row matmul),
keeping the serial chain per step minimal:
  rec-MM (16 bf16 matmuls) -> sigmoid/tanh (ACT) -> cell update (DVE) ->
  tanh(c) (ACT) -> h write (DVE, bf16) -> next rec-MM.

The embedding gather + PE transposes for the input tokens are pipelined into
the first recurrence steps (only the two tiles needed by block 0 are done up
front), so the recurrence starts ~13us into the kernel instead of ~43us.
"""

import numpy as np

B, T_FULL, V, E, H = 64, 256, 32000, 256, 256
G = 4 * H            # 1024 gate dim
NCORE = 8
BL = B // NCORE      # 8 batch rows per core
SB = 8               # recurrence steps per PSUM block

_CACHE = {}


def _emit(nc, tc, d, T):
    """Emit the whole kernel under TileContext tc. d = dict of dram handles."""
    from concourse import bass, mybir
    from concourse.masks import make_identity

    f32 = mybir.dt.float32
    bf16 = mybir.dt.bfloat16
    AF = mybir.ActivationFunctionType
    OP = mybir.AluOpType

    NTOK = BL * T
    NTILE = NTOK // 128
    NBLK = T // SB
    L2 = 2 * NTILE       # logits tile free dim (classes packed)

    persist = tc.alloc_tile_pool(name="persist", bufs=1)

    # ---------------- persistent SBUF buffers ----------------
    idx_sb = persist.tile([128, NTILE], mybir.dt.int32, tag="idx", name="idx")
    ident = persist.tile([128, 128], f32, tag="ident", name="ident")
    xT = persist.tile([128, 2 * NTOK], bf16, tag="xT", name="xT")  # [p, k(2), tok]
    wx = [persist.tile([128, 2048], bf16, tag=f"wx{dd}", name=f"wx{dd}") for dd in range(2)]
    wh = [persist.tile([128, 2048], bf16, tag=f"wh{dd}", name=f"wh{dd}") for dd in range(2)]
    bias16 = [persist.tile([1, G], bf16, tag=f"bias16_{dd}", name=f"bias16_{dd}")
              for dd in range(2)]
    hst = [persist.tile([128, 16 * T], bf16, tag=f"h{dd}", name=f"h{dd}") for dd in range(2)]
    ones64 = persist.tile([1, 64], bf16, tag="ones64", name="ones64")
    w_out = persist.tile([128, 8], bf16, tag="w_out", name="w_out")
    lg_sb = persist.tile([128, L2], f32, tag="lg_sb", name="lg_sb")

    # ---------------- constants (gpsimd, ahead of the gathers) -------------
    make_identity(nc, ident[:])
    nc.gpsimd.memset(ones64[:], 1.0)

    # ---------------- weight / index loads (sync + scalar queues) ----------
    nc.sync.dma_start(idx_sb[:], d["idx"].ap())
    nc.sync.dma_start(wx[0][:], d["wx"].ap()[0])
    nc.sync.dma_start(wh[0][:], d["wh"].ap()[0])
    nc.scalar.dma_start(wx[1][:], d["wx"].ap()[1])
    nc.scalar.dma_start(wh[1][:], d["wh"].ap()[1])
    for dd in range(2):
        nc.sync.dma_start(bias16[dd][:], d["bias"].ap()[dd : dd + 1])
    nc.scalar.dma_start(w_out[:], d["wout"].ap())

    # ---------------- stage A: gather + transpose ----------------
    xTr = xT[:].rearrange("p (k n) -> p k n", k=2)
    # interleave from both ends: fw consumes tile 0 first, bw tile NTILE-1
    order = []
    for i in range((NTILE + 1) // 2):
        order.append(i)
        if NTILE - 1 - i > i:
            order.append(NTILE - 1 - i)
    pg = tc.alloc_tile_pool(name="gather", bufs=1)
    pps = tc.alloc_tile_pool(name="tps", bufs=2, space="PSUM")
    # queue every gather immediately (they serialize on the gpsimd queue)
    xgs = {}
    for i in order:
        xg = pg.tile([128, E], f32, tag=f"xg{i}", name=f"xg{i}")
        xgs[i] = xg
        nc.gpsimd.indirect_dma_start(
            out=xg[:], out_offset=None, in_=d["emb"].ap(),
            in_offset=bass.IndirectOffsetOnAxis(ap=idx_sb[:, i : i + 1], axis=0),
        )

    def tile_ops(i):
        """Closures emitting the PE transpose + bf16 cast for tile i."""
        ops = []
        for k in range(2):
            def t_op(i=i, k=k):
                ps = pps.tile([128, 128], f32, tag="tp", name="tp")
                mm = nc.tensor.transpose(out=ps[:], in_=xgs[i][:, k * 128 : (k + 1) * 128],
                                         identity=ident[:])
                nc.vector.tensor_copy(xTr[:, k, i * 128 : (i + 1) * 128], ps[:])
                return mm
            ops.append(t_op)
        return ops

    # only the tiles block 0 needs are transposed up front; the rest are
    # pinned into the PE idle windows of the first recurrence steps
    for i in order[:2]:
        for op in tile_ops(i):
            op()
    bgq = []
    for i in order[2:]:
        bgq.extend(tile_ops(i))

    # ---------------- recurrence ----------------
    # hst layout: [p, k(2), t(T), b(8)]  (k-major so the loss-stage stationary
    # operand over tokens is a single contiguous free dim)
    hr = [hst[dd][:].rearrange("p (k t b) -> p k t b", k=2, b=8) for dd in range(2)]
    wxr = [wx[dd][:].rearrange("p (q j) -> p q j", j=128) for dd in range(2)]
    whr = [wh[dd][:].rearrange("p (q j) -> p q j", j=128) for dd in range(2)]

    def t0_of(dd, bi):
        return bi * SB if dd == 0 else T - SB - bi * SB

    zpool = [tc.alloc_tile_pool(name=f"z{dd}", bufs=2, space="PSUM")
             for dd in range(2)]
    ztile = [{}, {}]

    def prefill_ops(dd, bi):
        """Closures emitting x-proj + bias matmuls for block bi of dir dd."""
        zt = zpool[dd].tile([128, SB * 64], f32, tag=f"zt{dd}", name=f"zt{dd}")
        ztile[dd][bi] = zt
        # m-major: col = m*64 + l*8 + b -> x-proj/bias matmuls write contiguous
        # column ranges (strided PSUM out-APs measured ~7x slower per matmul)
        ztr = zt[:].rearrange("p (m l b) -> p m l b", l=SB, m=8, b=8)
        t0 = t0_of(dd, bi)
        # local index of global step s=0 in this block (block 0 only);
        # that region gets no recurrent matmul, so bias must close the group
        l_s0 = None
        if bi == 0:
            l_s0 = (0 - t0) if dd == 0 else (T - 1 - t0)
        ops = []
        for m in range(8):
            for k in range(2):
                def op_x(m=m, k=k):
                    return nc.tensor.matmul(
                        out=ztr[:, m, :, :],
                        lhsT=wxr[dd][:, k * 8 + m, :],
                        rhs=xTr[:, k, t0 * 8 : (t0 + SB) * 8],
                        start=(k == 0), stop=False)
                ops.append(op_x)

            def op_b(m=m, l_s0=l_s0):
                if l_s0 is None:
                    return nc.tensor.matmul(out=ztr[:, m, :, :],
                                     lhsT=bias16[dd][:, m * 128 : (m + 1) * 128],
                                     rhs=ones64[:, 0 : SB * 8],
                                     start=False, stop=False)
                else:
                    rest = slice(1, SB) if l_s0 == 0 else slice(0, SB - 1)
                    nc.tensor.matmul(out=ztr[:, m, rest, :],
                                     lhsT=bias16[dd][:, m * 128 : (m + 1) * 128],
                                     rhs=ones64[:, 0 : (SB - 1) * 8],
                                     start=False, stop=False)
                    return nc.tensor.matmul(out=ztr[:, m, l_s0, :],
                                     lhsT=bias16[dd][:, m * 128 : (m + 1) * 128],
                                     rhs=ones64[:, 0:8],
                                     start=False, stop=True)
            ops.append(op_b)
        return ops

    gp = tc.alloc_tile_pool(name="gates", bufs=6)

    # rolling per-step work tiles: cols 0:64 = sigmoid(gates) [o,i,f,j2]
    # written at step s, cols 64:80 = c written by step s-1.  Fresh pool tile
    # per step keeps every write single-assignment (no per-step cross-engine
    # WAR semaphores on a persistent tile).
    cur_w = [None, None]
    for dd in range(2):
        w0 = gp.tile([128, 80], f32, tag=f"wk{dd}", name=f"wk{dd}")
        nc.gpsimd.memset(w0[:, 64:80], 0.0)
        cur_w[dd] = w0

    def step(dd, s):
        bi = s // SB
        t = s if dd == 0 else T - 1 - s
        l = t - t0_of(dd, bi)
        zt = ztile[dd][bi]
        ztr = zt[:].rearrange("p (m l b) -> p m l b", l=SB, m=8, b=8)
        rec_first = rec_last = None
        if s > 0:
            tp = t - 1 if dd == 0 else t + 1
            for m in range(8):
                for k in range(2):
                    mm = nc.tensor.matmul(out=ztr[:, m, l, :],
                                          lhsT=whr[dd][:, k * 8 + m, :],
                                          rhs=hr[dd][:, k, tp, :],
                                          start=False, stop=(k == 1))
                    rec_last = mm
                    if rec_first is None:
                        rec_first = mm
        w = cur_w[dd]
        nxt = gp.tile([128, 80], f32, tag=f"wk{dd}", name=f"wk{dd}")
        cur_w[dd] = nxt
        # one sigmoid over all four gates [o,i,f,j2]; j-weights were doubled
        # so sig_j2 = sigmoid(2j) and tanh(j) = 2*sig_j2 - 1
        nc.scalar.activation(w[:, 0:64].rearrange("p (m b) -> p m b", b=8),
                             ztr[:, :, l, :], AF.Sigmoid)
        # paired product: [sig_i*sig_j2 | sig_f*c] in one op
        pm = gp.tile([128, 32], f32, tag="pm", name="pm")
        nc.vector.tensor_tensor(pm[:], w[:, 16:48], w[:, 48:80], op=OP.mult)
        # v = 2*sig_i*sig_j2 - sig_i = sig_i * tanh(j)
        vt = gp.tile([128, 16], f32, tag="vt", name="vt")
        nc.vector.scalar_tensor_tensor(out=vt[:], in0=pm[:, 0:16], scalar=2.0,
                                       in1=w[:, 16:32], op0=OP.mult,
                                       op1=OP.subtract)
        # c = sig_f*c + sig_i*tanh(j), written into the NEXT step's work tile
        nc.vector.tensor_tensor(nxt[:, 64:80], vt[:], pm[:, 16:32], op=OP.add)
        tct = gp.tile([128, 16], f32, tag="tct", name="tct")
        nc.scalar.activation(tct[:], nxt[:, 64:80], AF.Tanh)
        nc.vector.tensor_tensor(hr[dd][:, :, t, :],
                                w[:, 0:16].rearrange("p (k b) -> p k b", k=2),
                                tct[:].rearrange("p (k b) -> p k b", k=2),
                                op=OP.mult)
        return rec_first, rec_last

    for op in prefill_ops(0, 0):
        op()
    for op in prefill_ops(1, 0):
        op()
    from concourse.tile_rust import add_dep_helper

    queues = [[], []]
    pending = []
    for s in range(T):
        if s % SB == 0:
            bi = s // SB
            for dd in range(2):
                queues[dd] = prefill_ops(dd, bi + 1) if bi + 1 < NBLK else []
        popped_all = []
        rec_f_first = rec_b_last = None
        for dd in range(2):
            for _ in range(3):
                if queues[dd]:
                    popped_all.append(queues[dd].pop(0)())
            rf, rl = step(dd, s)
            if dd == 0:
                rec_f_first = rf
            else:
                rec_b_last = rl
        # background gather-transposes ride the same pinning as prefill
        for _ in range(2):
            if bgq:
                popped_all.append(bgq.pop(0)())
        # pin prefill into the inter-step PE idle window: after BOTH dirs'
        # recurrent matmuls of this step, before the next step's first
        if rec_f_first is not None:
            for pi in pending:
                add_dep_helper(rec_f_first.ins, pi.ins, sync=False,
                               reason="prefill before next-step rec")
        if rec_b_last is not None:
            for pi in popped_all:
                add_dep_helper(pi.ins, rec_b_last.ins, sync=False,
                               reason="prefill after this-step rec")
            pending = popped_all
        else:
            pending = pending + popped_all
    for dd in range(2):
        for op in queues[dd]:
            op()
    for op in bgq:
        op()

    # ---------------- output projection ----------------
    with tc.tile_pool(name="lps", bufs=1, space="PSUM") as plp:
        lg = plp.tile([128, L2], f32, tag="lg", name="lg")
        for ti in range(NTILE):
            for kk in range(4):
                dd, ch = kk // 2, kk % 2
                nc.tensor.matmul(
                    out=lg[:, ti * 2 : ti * 2 + 2],
                    lhsT=hst[dd][:, ch * T * 8 + ti * 128 :
                                  ch * T * 8 + (ti + 1) * 128],
                    rhs=w_out[:, kk * 2 : kk * 2 + 2],
                    start=(kk == 0), stop=(kk == 3))
        nc.vector.tensor_copy(lg_sb[:], lg[:])
    nc.sync.dma_start(d["logits"].ap(), lg_sb[:])
    gp.release()
    zpool[1].release()
    zpool[0].release()
    pps.release()
    pg.release()
    persist.release()


def _build(T=T_FULL):
    if T in _CACHE:
        return _CACHE[T]
    from concourse import bacc, mybir, tile

    f32 = mybir.dt.float32
    bf16 = mybir.dt.bfloat16
    nc = bacc.Bacc("TRN2", target_bir_lowering=False, debug=False,
                   enable_asserts=False, num_devices=NCORE)
    NTOK = BL * T
    NTILE = NTOK // 128
    d = {
        "idx": nc.dram_tensor("idx", [128, NTILE], mybir.dt.int32,
                              kind="ExternalInput"),
        "emb": nc.dram_tensor("emb", [V, E], f32, kind="ExternalInput"),
        "wx": nc.dram_tensor("wx", [2, 128, 2048], bf16, kind="ExternalInput"),
        "wh": nc.dram_tensor("wh", [2, 128, 2048], bf16, kind="ExternalInput"),
        "bias": nc.dram_tensor("bias", [2, G], bf16, kind="ExternalInput"),
        "wout": nc.dram_tensor("wout", [128, 8], bf16, kind="ExternalInput"),
        "logits": nc.dram_tensor("logits", [128, 2 * NTILE], f32,
                                 kind="ExternalOutput"),
    }
    with tile.TileContext(nc) as tc:
        _emit(nc, tc, d, T)
    nc.compile()
    _CACHE[T] = (nc, d)
    return nc, d


GATE_PERM = np.r_[768:1024, 0:256, 512:768, 256:512]   # [o, i, f, j]


def _stage_core(core, inputs, outputs, mask, emb, Wk_f, b_f, Wk_b, b_b, W, b, T):
    """Build the per-core input map (pure slicing / transposition / layout)."""
    import ml_dtypes

    k8 = core * BL
    NTOK = BL * T
    NTILE = NTOK // 128
    idx = np.ascontiguousarray(
        inputs[k8 : k8 + BL, :T].T.reshape(NTOK).reshape(NTILE, 128).T
    ).astype(np.int32)
    bf = ml_dtypes.bfloat16
    wx = np.empty((2, 128, 2048), bf)
    wh = np.empty((2, 128, 2048), bf)
    bias = np.empty((2, G), bf)
    for dd, (Wk, bb) in enumerate(((Wk_f, b_f), (Wk_b, b_b))):
        Wp = np.asarray(Wk, np.float32)[:, GATE_PERM].copy()
        bp = np.asarray(bb, np.float32)[GATE_PERM].copy()
        # TF LSTMCell forget bias (permuted order o,i,f,j -> f at 512:768)
        bp[512:768] += 1.0
        # tanh(j) = 2*sigmoid(2j)-1: double the j-gate weights and bias so the
        # one big sigmoid op covers j too (x2 is exact in bf16)
        Wp[:, 768:1024] *= 2.0
        bp[768:1024] *= 2.0
        wx[dd] = (Wp[:E].reshape(2, 128, 8, 128).transpose(1, 0, 2, 3)
                  .reshape(128, 2048).astype(bf))
        wh[dd] = (Wp[E:].reshape(2, 128, 8, 128).transpose(1, 0, 2, 3)
                  .reshape(128, 2048).astype(bf))
        bias[dd] = bp.astype(bf)
    wout = W.reshape(4, 128, 2).transpose(1, 0, 2).reshape(128, 8).astype(bf)
    return {
        "idx": idx,
        "emb": np.asarray(emb, np.float32),
        "wx": wx, "wh": wh, "bias": bias,
        "wout": np.ascontiguousarray(wout),
    }


def run(inputs, outputs, mask, emb, Wk_f, b_f, Wk_b, b_b, W, b,
        T=T_FULL, trace=False):
    from concourse import bass_utils

    nc, d = _build(T)
    args = (np.asarray(inputs), np.asarray(outputs, np.float32),
            np.asarray(mask, np.float32), np.asarray(emb, np.float32),
            np.asarray(Wk_f, np.float32), np.asarray(b_f, np.float32),
            np.asarray(Wk_b, np.float32), np.asarray(b_b, np.float32),
            np.asarray(W, np.float32), np.asarray(b, np.float32))
    in_maps = [_stage_core(kc, *args, T) for kc in range(NCORE)]
    res = bass_utils.run_bass_kernel_spmd(nc, in_maps, core_ids=list(range(NCORE)),
                                          trace=trace)
    NTOK = BL * T
    NTILE = NTOK // 128
    # host tail: assemble logits, add b, float64 log-softmax xent, mean
    logits = np.empty((B, T, 2), np.float64)
    for kc in range(NCORE):
        lo = np.asarray(res.results[kc]["logits"], np.float64)   # [128, 2*NTILE]
        lo = lo.reshape(128, NTILE, 2).transpose(1, 0, 2).reshape(NTOK, 2)
        logits[kc * BL : (kc + 1) * BL] = lo.reshape(T, BL, 2).transpose(1, 0, 2)
    logits += np.asarray(b, np.float64)
    m = logits.max(-1, keepdims=True)
    lsm = logits - (m + np.log(np.exp(logits - m).sum(-1, keepdims=True)))
    xent = -(np.asarray(outputs, np.float64)[:, :T] * lsm).sum(-1)
    loss = np.float32((xent * np.asarray(mask, np.float64)[:, :T]).mean())
    return np.asarray(loss), res


def kernel(inputs, outputs, mask, emb, Wk_f, b_f, Wk_b, b_b, W, b):
    loss, _ = run(inputs, outputs, mask, emb, Wk_f, b_f, Wk_b, b_b, W, b)
    return loss


# revision 14
# speedup vs baseline: 1.1508x; 1.1508x over previous
"""Bass/Trainium2 kernel for a BiLSTM (TF-LSTMCell) cross-entropy loss.

Model (per reference):
  x = emb[inputs]                        # [B,T,E]
  h_fw = LSTM(x, Wk_f, b_f)              # forward over T
  h_bw = reverse(LSTM(reverse(x), Wk_b, b_b))
  logits = concat(h_fw, h_bw) @ W + b    # [B,T,2]
  loss = mean(xent(logits, outputs) * mask)

Sharding: data-parallel over batch. B=64 -> 8 cores x 8 rows.  Each core runs
both LSTM directions (two independent chains interleaved to hide latency) and
produces the pre-bias logits for its tokens; the host adds b, computes the
log-softmax cross entropy in float64 and averages (same split as summing the 8
per-core partials: the device does all O(B*T*H) work, the host the O(B*T) tail,
which also avoids an ACT-table swap for LN on device).

Device layout is feature-major: gate/feature index on the SBUF partition axis,
(time x batch) on the free axis, so per-step elementwise work is [128, small]
and the recurrent matmul keeps the weight stationary (bf16 -> fast weight
load).  z pre-activations accumulate in PSUM in 8-step blocks:
  psum col = m*64 + l*8 + b   (l=step-in-block, m=gate chunk of 128, b=batch)
Gate order is host-permuted to [o, i, f, j] so sigmoid covers one contiguous
[128,48] slice and tanh(j) one [128,16].  Weight prep (forget bias +1, j-gate
x2 for the tanh-via-sigmoid trick, bf16 cast, permutation) happens on the host
so weights are DMA-ready.  x-projection and bias are injected into each PSUM
block ahead of time by large-N matmuls (bias via a K=1 ones-row matmul),
keeping the serial chain per step minimal:
  rec-MM (16 bf16 matmuls) -> sigmoid/tanh (ACT) -> cell update (DVE) ->
  tanh(c) (ACT) -> h write (DVE, bf16) -> next rec-MM.

The embedding gather + PE transposes for the input tokens are pipelined into
the first recurrence steps (only the two tiles needed by block 0 are done up
front), so the recurrence starts ~13us into the kernel instead of ~43us.
"""

import numpy as np

B, T_FULL, V, E, H = 64, 256, 32000, 256, 256
G = 4 * H            # 1024 gate dim
NCORE = 8
BL = B // NCORE      # 8 batch rows per core
SB = 8               # recurrence steps per PSUM block

_CACHE = {}


def _emit(nc, tc, d, T):
    """Emit the whole kernel under TileContext tc. d = dict of dram handles."""
    from concourse import bass, mybir

    f32 = mybir.dt.float32
    bf16 = mybir.dt.bfloat16
    AF = mybir.ActivationFunctionType
    OP = mybir.AluOpType

    NTOK = BL * T
    NTILE = NTOK // 128
    NBLK = T // SB
    L2 = 2 * NTILE       # logits tile free dim (classes packed)

    persist = tc.alloc_tile_pool(name="persist", bufs=1)

    # ---------------- persistent SBUF buffers ----------------
    idx_sb = persist.tile([128, NTILE], mybir.dt.int32, tag="idx", name="idx")
    xT = persist.tile([128, 2 * NTOK], bf16, tag="xT", name="xT")  # [p, k(2), tok]
    wx = [persist.tile([128, 2048], bf16, tag=f"wx{dd}", name=f"wx{dd}") for dd in range(2)]
    wh = [persist.tile([128, 2048], bf16, tag=f"wh{dd}", name=f"wh{dd}") for dd in range(2)]
    bias16 = [persist.tile([1, G], bf16, tag=f"bias16_{dd}", name=f"bias16_{dd}")
              for dd in range(2)]
    hst = [persist.tile([128, 16 * T], bf16, tag=f"h{dd}", name=f"h{dd}") for dd in range(2)]
    ones64 = persist.tile([1, 64], bf16, tag="ones64", name="ones64")
    w_out = persist.tile([128, 8], bf16, tag="w_out", name="w_out")
    lg_sb = persist.tile([128, L2], f32, tag="lg_sb", name="lg_sb")

    # ---------------- constants (gpsimd, ahead of the gathers) -------------
    nc.gpsimd.memset(ones64[:], 1.0)

    # ---------------- weight / index loads (sync + scalar queues) ----------
    nc.sync.dma_start(idx_sb[:], d["idx"].ap())
    nc.sync.dma_start(wx[0][:], d["wx"].ap()[0])
    nc.sync.dma_start(wh[0][:], d["wh"].ap()[0])
    nc.scalar.dma_start(wx[1][:], d["wx"].ap()[1])
    nc.scalar.dma_start(wh[1][:], d["wh"].ap()[1])
    for dd in range(2):
        nc.sync.dma_start(bias16[dd][:], d["bias"].ap()[dd : dd + 1])
    nc.scalar.dma_start(w_out[:], d["wout"].ap())

    # ---------------- stage A: gather + xbar transpose ----------------
    # bf16 embedding rows are gathered per 128-token tile (gpsimd SWDGE),
    # then transposed feature-major entirely on the DMA xbar (sync queue) --
    # no PE/ACT/DVE involvement, so stage A never perturbs the recurrence.
    xTr = xT[:].rearrange("p (k n) -> p k n", k=2)
    # interleave from both ends: fw consumes tile 0 first, bw tile NTILE-1
    order = []
    for i in range((NTILE + 1) // 2):
        order.append(i)
        if NTILE - 1 - i > i:
            order.append(NTILE - 1 - i)
    pg = tc.alloc_tile_pool(name="gather", bufs=1)
    # queue every gather immediately (they serialize on the gpsimd queue)
    xgs = {}
    for i in order:
        xg = pg.tile([128, E], bf16, tag=f"xg{i}", name=f"xg{i}")
        xgs[i] = xg
        nc.gpsimd.indirect_dma_start(
            out=xg[:], out_offset=None, in_=d["emb"].ap(),
            in_offset=bass.IndirectOffsetOnAxis(ap=idx_sb[:, i : i + 1], axis=0),
        )
    for i in order:
        for k in range(2):
            nc.sync.dma_start_transpose(
                out=xTr[:, k, i * 128 : (i + 1) * 128],
                in_=xgs[i][:, k * 128 : (k + 1) * 128])

    # ---------------- recurrence ----------------
    # hst layout: [p, k(2), t(T), b(8)]  (k-major so the loss-stage stationary
    # operand over tokens is a single contiguous free dim)
    hr = [hst[dd][:].rearrange("p (k t b) -> p k t b", k=2, b=8) for dd in range(2)]
    wxr = [wx[dd][:].rearrange("p (q j) -> p q j", j=128) for dd in range(2)]
    whr = [wh[dd][:].rearrange("p (q j) -> p q j", j=128) for dd in range(2)]

    def t0_of(dd, bi):
        return bi * SB if dd == 0 else T - SB - bi * SB

    zpool = [tc.alloc_tile_pool(name=f"z{dd}", bufs=2, space="PSUM")
             for dd in range(2)]
    ztile = [{}, {}]

    def prefill_ops(dd, bi):
        """Closures emitting x-proj + bias matmuls for block bi of dir dd."""
        zt = zpool[dd].tile([128, SB * 64], f32, tag=f"zt{dd}", name=f"zt{dd}")
        ztile[dd][bi] = zt
        # m-major: col = m*64 + l*8 + b -> x-proj/bias matmuls write contiguous
        # column ranges (strided PSUM out-APs measured ~7x slower per matmul)
        ztr = zt[:].rearrange("p (m l b) -> p m l b", l=SB, m=8, b=8)
        t0 = t0_of(dd, bi)
        # local index of global step s=0 in this block (block 0 only);
        # that region gets no recurrent matmul, so bias must close the group
        l_s0 = None
        if bi == 0:
            l_s0 = (0 - t0) if dd == 0 else (T - 1 - t0)
        ops = []
        for m in range(8):
            for k in range(2):
                def op_x(m=m, k=k):
                    return nc.tensor.matmul(
                        out=ztr[:, m, :, :],
                        lhsT=wxr[dd][:, k * 8 + m, :],
                        rhs=xTr[:, k, t0 * 8 : (t0 + SB) * 8],
                        start=(k == 0), stop=False)
                ops.append(op_x)

            def op_b(m=m, l_s0=l_s0):
                if l_s0 is None:
                    return nc.tensor.matmul(out=ztr[:, m, :, :],
                                     lhsT=bias16[dd][:, m * 128 : (m + 1) * 128],
                                     rhs=ones64[:, 0 : SB * 8],
                                     start=False, stop=False)
                else:
                    rest = slice(1, SB) if l_s0 == 0 else slice(0, SB - 1)
                    nc.tensor.matmul(out=ztr[:, m, rest, :],
                                     lhsT=bias16[dd][:, m * 128 : (m + 1) * 128],
                                     rhs=ones64[:, 0 : (SB - 1) * 8],
                                     start=False, stop=False)
                    return nc.tensor.matmul(out=ztr[:, m, l_s0, :],
                                     lhsT=bias16[dd][:, m * 128 : (m + 1) * 128],
                                     rhs=ones64[:, 0:8],
                                     start=False, stop=True)
            ops.append(op_b)
        return ops

    gp = tc.alloc_tile_pool(name="gates", bufs=6)

    # rolling per-step work tiles: cols 0:64 = sigmoid(gates) [o,i,f,j2]
    # written at step s, cols 64:80 = c written by step s-1.  Fresh pool tile
    # per step keeps every write single-assignment (no per-step cross-engine
    # WAR semaphores on a persistent tile).
    cur_w = [None, None]
    for dd in range(2):
        w0 = gp.tile([128, 80], f32, tag=f"wk{dd}", name=f"wk{dd}")
        nc.gpsimd.memset(w0[:, 64:80], 0.0)
        cur_w[dd] = w0

    def step(dd, s):
        bi = s // SB
        t = s if dd == 0 else T - 1 - s
        l = t - t0_of(dd, bi)
        zt = ztile[dd][bi]
        ztr = zt[:].rearrange("p (m l b) -> p m l b", l=SB, m=8, b=8)
        rec_first = rec_last = None
        if s > 0:
            tp = t - 1 if dd == 0 else t + 1
            for m in range(8):
                for k in range(2):
                    mm = nc.tensor.matmul(out=ztr[:, m, l, :],
                                          lhsT=whr[dd][:, k * 8 + m, :],
                                          rhs=hr[dd][:, k, tp, :],
                                          start=False, stop=(k == 1))
                    rec_last = mm
                    if rec_first is None:
                        rec_first = mm
        w = cur_w[dd]
        nxt = gp.tile([128, 80], f32, tag=f"wk{dd}", name=f"wk{dd}")
        cur_w[dd] = nxt
        # one sigmoid over all four gates [o,i,f,j2]; j-weights were doubled
        # so sig_j2 = sigmoid(2j) and tanh(j) = 2*sig_j2 - 1
        nc.scalar.activation(w[:, 0:64].rearrange("p (m b) -> p m b", b=8),
                             ztr[:, :, l, :], AF.Sigmoid)
        # paired product: [sig_i*sig_j2 | sig_f*c] in one op
        pm = gp.tile([128, 32], f32, tag="pm", name="pm")
        nc.vector.tensor_tensor(pm[:], w[:, 16:48], w[:, 48:80], op=OP.mult)
        # v = 2*sig_i*sig_j2 - sig_i = sig_i * tanh(j)
        vt = gp.tile([128, 16], f32, tag="vt", name="vt")
        nc.vector.scalar_tensor_tensor(out=vt[:], in0=pm[:, 0:16], scalar=2.0,
                                       in1=w[:, 16:32], op0=OP.mult,
                                       op1=OP.subtract)
        # c = sig_f*c + sig_i*tanh(j), written into the NEXT step's work tile
        nc.vector.tensor_tensor(nxt[:, 64:80], vt[:], pm[:, 16:32], op=OP.add)
        tct = gp.tile([128, 16], f32, tag="tct", name="tct")
        nc.scalar.activation(tct[:], nxt[:, 64:80], AF.Tanh)
        nc.vector.tensor_tensor(hr[dd][:, :, t, :],
                                w[:, 0:16].rearrange("p (k b) -> p k b", k=2),
                                tct[:].rearrange("p (k b) -> p k b", k=2),
                                op=OP.mult)
        return rec_first, rec_last

    for op in prefill_ops(0, 0):
        op()
    for op in prefill_ops(1, 0):
        op()
    from concourse.tile_rust import add_dep_helper

    queues = [[], []]
    pending = []
    for s in range(T):
        if s % SB == 0:
            bi = s // SB
            for dd in range(2):
                queues[dd] = prefill_ops(dd, bi + 1) if bi + 1 < NBLK else []
        popped_all = []
        rec_f_first = rec_b_last = None
        for dd in range(2):
            for _ in range(3):
                if queues[dd]:
                    popped_all.append(queues[dd].pop(0)())
            rf, rl = step(dd, s)
            if dd == 0:
                rec_f_first = rf
            else:
                rec_b_last = rl
        # pin prefill into the inter-step PE idle window: after BOTH dirs'
        # recurrent matmuls of this step, before the next step's first
        if rec_f_first is not None:
            for pi in pending:
                add_dep_helper(rec_f_first.ins, pi.ins, sync=False,
                               reason="prefill before next-step rec")
        if rec_b_last is not None:
            for pi in popped_all:
                add_dep_helper(pi.ins, rec_b_last.ins, sync=False,
                               reason="prefill after this-step rec")
            pending = popped_all
        else:
            pending = pending + popped_all
    for dd in range(2):
        for op in queues[dd]:
            op()

    # ---------------- output projection ----------------
    with tc.tile_pool(name="lps", bufs=1, space="PSUM") as plp:
        lg = plp.tile([128, L2], f32, tag="lg", name="lg")
        for ti in range(NTILE):
            for kk in range(4):
                dd, ch = kk // 2, kk % 2
                nc.tensor.matmul(
                    out=lg[:, ti * 2 : ti * 2 + 2],
                    lhsT=hst[dd][:, ch * T * 8 + ti * 128 :
                                  ch * T * 8 + (ti + 1) * 128],
                    rhs=w_out[:, kk * 2 : kk * 2 + 2],
                    start=(kk == 0), stop=(kk == 3))
        nc.vector.tensor_copy(lg_sb[:], lg[:])
    nc.sync.dma_start(d["logits"].ap(), lg_sb[:])
    gp.release()
    zpool[1].release()
    zpool[0].release()
    pg.release()
    persist.release()


def _build(T=T_FULL):
    if T in _CACHE:
        return _CACHE[T]
    from concourse import bacc, mybir, tile

    f32 = mybir.dt.float32
    bf16 = mybir.dt.bfloat16
    nc = bacc.Bacc("TRN2", target_bir_lowering=False, debug=False,
                   enable_asserts=False, num_devices=NCORE)
    NTOK = BL * T
    NTILE = NTOK // 128
    d = {
        "idx": nc.dram_tensor("idx", [128, NTILE], mybir.dt.int32,
                              kind="ExternalInput"),
        "emb": nc.dram_tensor("emb", [V, E], bf16, kind="ExternalInput"),
        "wx": nc.dram_tensor("wx", [2, 128, 2048], bf16, kind="ExternalInput"),
        "wh": nc.dram_tensor("wh", [2, 128, 2048], bf16, kind="ExternalInput"),
        "bias": nc.dram_tensor("bias", [2, G], bf16, kind="ExternalInput"),
        "wout": nc.dram_tensor("wout", [128, 8], bf16, kind="ExternalInput"),
        "logits": nc.dram_tensor("logits", [128, 2 * NTILE], f32,
                                 kind="ExternalOutput"),
    }
    with tile.TileContext(nc) as tc:
        _emit(nc, tc, d, T)
    nc.compile()
    _CACHE[T] = (nc, d)
    return nc, d


GATE_PERM = np.r_[768:1024, 0:256, 512:768, 256:512]   # [o, i, f, j]


def _stage_core(core, inputs, outputs, mask, emb16, Wk_f, b_f, Wk_b, b_b, W, b, T):
    """Build the per-core input map (pure slicing / transposition / layout).
    emb16 is the embedding table already cast to bf16 (shared across cores)."""
    import ml_dtypes

    k8 = core * BL
    NTOK = BL * T
    NTILE = NTOK // 128
    idx = np.ascontiguousarray(
        inputs[k8 : k8 + BL, :T].T.reshape(NTOK).reshape(NTILE, 128).T
    ).astype(np.int32)
    bf = ml_dtypes.bfloat16
    wx = np.empty((2, 128, 2048), bf)
    wh = np.empty((2, 128, 2048), bf)
    bias = np.empty((2, G), bf)
    for dd, (Wk, bb) in enumerate(((Wk_f, b_f), (Wk_b, b_b))):
        Wp = np.asarray(Wk, np.float32)[:, GATE_PERM].copy()
        bp = np.asarray(bb, np.float32)[GATE_PERM].copy()
        # TF LSTMCell forget bias (permuted order o,i,f,j -> f at 512:768)
        bp[512:768] += 1.0
        # tanh(j) = 2*sigmoid(2j)-1: double the j-gate weights and bias so the
        # one big sigmoid op covers j too (x2 is exact in bf16)
        Wp[:, 768:1024] *= 2.0
        bp[768:1024] *= 2.0
        wx[dd] = (Wp[:E].reshape(2, 128, 8, 128).transpose(1, 0, 2, 3)
                  .reshape(128, 2048).astype(bf))
        wh[dd] = (Wp[E:].reshape(2, 128, 8, 128).transpose(1, 0, 2, 3)
                  .reshape(128, 2048).astype(bf))
        bias[dd] = bp.astype(bf)
    wout = W.reshape(4, 128, 2).transpose(1, 0, 2).reshape(128, 8).astype(bf)
    return {
        "idx": idx,
        "emb": emb16,
        "wx": wx, "wh": wh, "bias": bias,
        "wout": np.ascontiguousarray(wout),
    }


def run(inputs, outputs, mask, emb, Wk_f, b_f, Wk_b, b_b, W, b,
        T=T_FULL, trace=False):
    from concourse import bass_utils

    import ml_dtypes

    nc, d = _build(T)
    emb16 = np.ascontiguousarray(
        np.asarray(emb, np.float32).astype(ml_dtypes.bfloat16))
    args = (np.asarray(inputs), np.asarray(outputs, np.float32),
            np.asarray(mask, np.float32), emb16,
            np.asarray(Wk_f, np.float32), np.asarray(b_f, np.float32),
            np.asarray(Wk_b, np.float32), np.asarray(b_b, np.float32),
            np.asarray(W, np.float32), np.asarray(b, np.float32))
    in_maps = [_stage_core(kc, *args, T) for kc in range(NCORE)]
    res = bass_utils.run_bass_kernel_spmd(nc, in_maps, core_ids=list(range(NCORE)),
                                          trace=trace)
    NTOK = BL * T
    NTILE = NTOK // 128
    # host tail: assemble logits, add b, float64 log-softmax xent, mean
    logits = np.empty((B, T, 2), np.float64)
    for kc in range(NCORE):
        lo = np.asarray(res.results[kc]["logits"], np.float64)   # [128, 2*NTILE]
        lo = lo.reshape(128, NTILE, 2).transpose(1, 0, 2).reshape(NTOK, 2)
        logits[kc * BL : (kc + 1) * BL] = lo.reshape(T, BL, 2).transpose(1, 0, 2)
    logits += np.asarray(b, np.float64)
    m = logits.max(-1, keepdims=True)
    lsm = logits - (m + np.log(np.exp(logits - m).sum(-1, keepdims=True)))
    xent = -(np.asarray(outputs, np.float64)[:, :T] * lsm).sum(-1)
    loss = np.float32((xent * np.asarray(mask, np.float64)[:, :T]).mean())
    return np.asarray(loss), res


def kernel(inputs, outputs, mask, emb, Wk_f, b_f, Wk_b, b_b, W, b):
    loss, _ = run(inputs, outputs, mask, emb, Wk_f, b_f, Wk_b, b_b, W, b)
    return loss


# revision 17
# speedup vs baseline: 1.2095x; 1.0511x over previous
"""Bass/Trainium2 kernel for a BiLSTM (TF-LSTMCell) cross-entropy loss.

Model (per reference):
  x = emb[inputs]                        # [B,T,E]
  h_fw = LSTM(x, Wk_f, b_f)              # forward over T
  h_bw = reverse(LSTM(reverse(x), Wk_b, b_b))
  logits = concat(h_fw, h_bw) @ W + b    # [B,T,2]
  loss = mean(xent(logits, outputs) * mask)

Sharding: data-parallel over batch. B=64 -> 8 cores x 8 rows.  Each core runs
both LSTM directions (two independent chains interleaved to hide latency) and
produces the pre-bias logits for its tokens; the host adds b, computes the
log-softmax cross entropy in float64 and averages (same split as summing the 8
per-core partials: the device does all O(B*T*H) work, the host the O(B*T) tail,
which also avoids an ACT-table swap for LN on device).

Device layout is feature-major: gate/feature index on the SBUF partition axis,
(time x batch) on the free axis, so per-step elementwise work is [128, small]
and the recurrent matmul keeps the weight stationary (bf16 -> fast weight
load).  z pre-activations accumulate in PSUM in 8-step blocks:
  psum col = m*64 + l*8 + b   (l=step-in-block, m=gate chunk of 128, b=batch)
Gate order is host-permuted to [o, i, f, j] so sigmoid covers one contiguous
[128,48] slice and tanh(j) one [128,16].  Weight prep (forget bias +1, j-gate
x2 for the tanh-via-sigmoid trick, bf16 cast, permutation) happens on the host
so weights are DMA-ready.  x-projection and bias are injected into each PSUM
block ahead of time by large-N matmuls (bias via a K=1 ones-row matmul),
keeping the serial chain per step minimal:
  rec-MM (16 bf16 matmuls) -> sigmoid/tanh (ACT) -> cell update (DVE) ->
  tanh(c) (ACT) -> h write (DVE, bf16) -> next rec-MM.

The embedding gather + PE transposes for the input tokens are pipelined into
the first recurrence steps (only the two tiles needed by block 0 are done up
front), so the recurrence starts ~13us into the kernel instead of ~43us.
"""

import numpy as np

B, T_FULL, V, E, H = 64, 256, 32000, 256, 256
G = 4 * H            # 1024 gate dim
NCORE = 8
BL = B // NCORE      # 8 batch rows per core
SB = 8               # recurrence steps per PSUM block

_CACHE = {}


def _emit(nc, tc, d, T):
    """Emit the whole kernel under TileContext tc. d = dict of dram handles."""
    from concourse import bass, mybir

    f32 = mybir.dt.float32
    bf16 = mybir.dt.bfloat16
    AF = mybir.ActivationFunctionType
    OP = mybir.AluOpType

    NTOK = BL * T
    NTILE = NTOK // 128
    NBLK = T // SB
    L2 = 2 * NTILE       # logits tile free dim (classes packed)

    persist = tc.alloc_tile_pool(name="persist", bufs=1)

    # ---------------- persistent SBUF buffers ----------------
    idx_sb = persist.tile([128, NTILE], mybir.dt.int32, tag="idx", name="idx")
    xT = persist.tile([128, 2 * NTOK], bf16, tag="xT", name="xT")  # [p, k(2), tok]
    wx = [persist.tile([128, 2048], bf16, tag=f"wx{dd}", name=f"wx{dd}") for dd in range(2)]
    wh = [persist.tile([128, 2048], bf16, tag=f"wh{dd}", name=f"wh{dd}") for dd in range(2)]
    bias16 = [persist.tile([1, G], bf16, tag=f"bias16_{dd}", name=f"bias16_{dd}")
              for dd in range(2)]
    hst = [persist.tile([128, 16 * T], bf16, tag=f"h{dd}", name=f"h{dd}") for dd in range(2)]
    ones64 = persist.tile([1, 64], bf16, tag="ones64", name="ones64")
    w_out = persist.tile([128, 8], bf16, tag="w_out", name="w_out")
    lg_sb = persist.tile([128, L2], f32, tag="lg_sb", name="lg_sb")

    # ---------------- constants (gpsimd, ahead of the gathers) -------------
    nc.gpsimd.memset(ones64[:], 1.0)

    # ---------------- weight / index loads (sync + scalar queues) ----------
    nc.sync.dma_start(idx_sb[:], d["idx"].ap())
    nc.sync.dma_start(wx[0][:], d["wx"].ap()[0])
    nc.sync.dma_start(wh[0][:], d["wh"].ap()[0])
    nc.scalar.dma_start(wx[1][:], d["wx"].ap()[1])
    nc.scalar.dma_start(wh[1][:], d["wh"].ap()[1])
    for dd in range(2):
        nc.sync.dma_start(bias16[dd][:], d["bias"].ap()[dd : dd + 1])
    nc.scalar.dma_start(w_out[:], d["wout"].ap())

    # ---------------- stage A: gather + xbar transpose ----------------
    # bf16 embedding rows are gathered per 128-token tile (gpsimd SWDGE),
    # then transposed feature-major entirely on the DMA xbar (sync queue) --
    # no PE/ACT/DVE involvement, so stage A never perturbs the recurrence.
    xTr = xT[:].rearrange("p (k n) -> p k n", k=2)
    # interleave from both ends: fw consumes tile 0 first, bw tile NTILE-1
    order = []
    for i in range((NTILE + 1) // 2):
        order.append(i)
        if NTILE - 1 - i > i:
            order.append(NTILE - 1 - i)
    pg = tc.alloc_tile_pool(name="gather", bufs=1)
    xgs = {}

    def gather(i):
        xg = pg.tile([128, E], bf16, tag=f"xg{i}", name=f"xg{i}")
        xgs[i] = xg
        nc.gpsimd.indirect_dma_start(
            out=xg[:], out_offset=None, in_=d["emb"].ap(),
            in_offset=bass.IndirectOffsetOnAxis(ap=idx_sb[:, i : i + 1], axis=0),
        )

    def transpose(i):
        for k in range(2):
            nc.sync.dma_start_transpose(
                out=xTr[:, k, i * 128 : (i + 1) * 128],
                in_=xgs[i][:, k * 128 : (k + 1) * 128])

    # block 0's two tiles (gather + transpose) first — they gate the first
    # recurrence step; everything else trails on the gpsimd/sync queues
    for i in order[:2]:
        gather(i)
    for i in order[:2]:
        transpose(i)
    for i in order[2:]:
        gather(i)
    for i in order[2:]:
        transpose(i)

    # ---------------- recurrence ----------------
    # hst layout: [p, k(2), t(T), b(8)]  (k-major so the loss-stage stationary
    # operand over tokens is a single contiguous free dim)
    hr = [hst[dd][:].rearrange("p (k t b) -> p k t b", k=2, b=8) for dd in range(2)]
    wxr = [wx[dd][:].rearrange("p (q j) -> p q j", j=128) for dd in range(2)]
    whr = [wh[dd][:].rearrange("p (q j) -> p q j", j=128) for dd in range(2)]

    def t0_of(dd, bi):
        return bi * SB if dd == 0 else T - SB - bi * SB

    zpool = [tc.alloc_tile_pool(name=f"z{dd}", bufs=2, space="PSUM")
             for dd in range(2)]
    ztile = [{}, {}]

    def prefill_ops(dd, bi):
        """Closures emitting x-proj + bias matmuls for block bi of dir dd."""
        zt = zpool[dd].tile([128, SB * 64], f32, tag=f"zt{dd}", name=f"zt{dd}")
        ztile[dd][bi] = zt
        # m-major: col = m*64 + l*8 + b -> x-proj/bias matmuls write contiguous
        # column ranges (strided PSUM out-APs measured ~7x slower per matmul)
        ztr = zt[:].rearrange("p (m l b) -> p m l b", l=SB, m=8, b=8)
        t0 = t0_of(dd, bi)
        # local index of global step s=0 in this block (block 0 only);
        # that region gets no recurrent matmul, so bias must close the group
        l_s0 = None
        if bi == 0:
            l_s0 = (0 - t0) if dd == 0 else (T - 1 - t0)
        ops = []
        for m in range(8):
            for k in range(2):
                def op_x(m=m, k=k):
                    return nc.tensor.matmul(
                        out=ztr[:, m, :, :],
                        lhsT=wxr[dd][:, k * 8 + m, :],
                        rhs=xTr[:, k, t0 * 8 : (t0 + SB) * 8],
                        start=(k == 0), stop=False)
                ops.append(op_x)

            def op_b(m=m, l_s0=l_s0):
                if l_s0 is None:
                    return nc.tensor.matmul(out=ztr[:, m, :, :],
                                     lhsT=bias16[dd][:, m * 128 : (m + 1) * 128],
                                     rhs=ones64[:, 0 : SB * 8],
                                     start=False, stop=False)
                else:
                    rest = slice(1, SB) if l_s0 == 0 else slice(0, SB - 1)
                    nc.tensor.matmul(out=ztr[:, m, rest, :],
                                     lhsT=bias16[dd][:, m * 128 : (m + 1) * 128],
                                     rhs=ones64[:, 0 : (SB - 1) * 8],
                                     start=False, stop=False)
                    return nc.tensor.matmul(out=ztr[:, m, l_s0, :],
                                     lhsT=bias16[dd][:, m * 128 : (m + 1) * 128],
                                     rhs=ones64[:, 0:8],
                                     start=False, stop=True)
            ops.append(op_b)
        return ops

    gp = tc.alloc_tile_pool(name="gates", bufs=6)

    # rolling per-step work tiles: cols 0:64 = sigmoid(gates) [o,i,f,j2]
    # written at step s, cols 64:80 = c written by step s-1.  Fresh pool tile
    # per step keeps every write single-assignment (no per-step cross-engine
    # WAR semaphores on a persistent tile).
    cur_w = [None, None]
    for dd in range(2):
        w0 = gp.tile([128, 80], f32, tag=f"wk{dd}", name=f"wk{dd}")
        nc.gpsimd.memset(w0[:, 64:80], 0.0)
        cur_w[dd] = w0

    def step(dd, s):
        bi = s // SB
        t = s if dd == 0 else T - 1 - s
        l = t - t0_of(dd, bi)
        zt = ztile[dd][bi]
        ztr = zt[:].rearrange("p (m l b) -> p m l b", l=SB, m=8, b=8)
        rec_first = rec_last = None
        if s > 0:
            tp = t - 1 if dd == 0 else t + 1
            for m in range(8):
                for k in range(2):
                    mm = nc.tensor.matmul(out=ztr[:, m, l, :],
                                          lhsT=whr[dd][:, k * 8 + m, :],
                                          rhs=hr[dd][:, k, tp, :],
                                          start=False, stop=(k == 1))
                    rec_last = mm
                    if rec_first is None:
                        rec_first = mm
        w = cur_w[dd]
        nxt = gp.tile([128, 80], f32, tag=f"wk{dd}", name=f"wk{dd}")
        cur_w[dd] = nxt
        o = {"rec_first": rec_first, "rec_last": rec_last}
        # one sigmoid over all four gates [o,i,f,j2]; j-weights were doubled
        # so sig_j2 = sigmoid(2j) and tanh(j) = 2*sig_j2 - 1
        o["sig"] = nc.scalar.activation(
            w[:, 0:64].rearrange("p (m b) -> p m b", b=8),
            ztr[:, :, l, :], AF.Sigmoid)
        # paired product: [sig_i*sig_j2 | sig_f*c] in one op
        pm = gp.tile([128, 32], f32, tag="pm", name="pm")
        o["pm"] = nc.vector.tensor_tensor(pm[:], w[:, 16:48], w[:, 48:80],
                                          op=OP.mult)
        # v = 2*sig_i*sig_j2 - sig_i = sig_i * tanh(j)
        vt = gp.tile([128, 16], f32, tag="vt", name="vt")
        o["vt"] = nc.vector.scalar_tensor_tensor(
            out=vt[:], in0=pm[:, 0:16], scalar=2.0, in1=w[:, 16:32],
            op0=OP.mult, op1=OP.subtract)
        # c = sig_f*c + sig_i*tanh(j), written into the NEXT step's work tile
        o["add"] = nc.vector.tensor_tensor(nxt[:, 64:80], vt[:], pm[:, 16:32],
                                           op=OP.add)
        tct = gp.tile([128, 16], f32, tag="tct", name="tct")
        o["tanh"] = nc.scalar.activation(tct[:], nxt[:, 64:80], AF.Tanh)
        o["h"] = nc.vector.tensor_tensor(
            hr[dd][:, :, t, :],
            w[:, 0:16].rearrange("p (k b) -> p k b", k=2),
            tct[:].rearrange("p (k b) -> p k b", k=2),
            op=OP.mult)
        return o

    for op in prefill_ops(0, 0):
        op()
    for op in prefill_ops(1, 0):
        op()
    from concourse.tile_rust import add_dep_helper

    queues = [[], []]
    pending = []
    for s in range(T):
        if s % SB == 0:
            bi = s // SB
            for dd in range(2):
                queues[dd] = prefill_ops(dd, bi + 1) if bi + 1 < NBLK else []
        popped_all = []
        so = [None, None]
        for dd in range(2):
            for _ in range(3):
                if queues[dd]:
                    popped_all.append(queues[dd].pop(0)())
            so[dd] = step(dd, s)
        rec_f_first = so[0]["rec_first"]
        rec_b_last = so[1]["rec_last"]
        # pin the steady-state DVE/ACT interleave (fw offset half a period
        # ahead of bw): pmA vtA addA pmB vtB addB hA hB / sigA sigB tanA tanB.
        # Soft deps only — keeps the scheduler from coupling the two serial
        # chains in an order that stretches the step period.
        oa, ob = so
        add_dep_helper(ob["pm"].ins, oa["add"].ins, sync=False,
                       reason="dve interleave")
        add_dep_helper(oa["tanh"].ins, ob["sig"].ins, sync=False,
                       reason="act interleave")
        add_dep_helper(oa["h"].ins, ob["add"].ins, sync=False,
                       reason="dve interleave")
        add_dep_helper(ob["h"].ins, oa["h"].ins, sync=False,
                       reason="dve interleave")
        # pin prefill into the inter-step PE idle window: after BOTH dirs'
        # recurrent matmuls of this step, before the next step's first
        if rec_f_first is not None:
            for pi in pending:
                add_dep_helper(rec_f_first.ins, pi.ins, sync=False,
                               reason="prefill before next-step rec")
        if rec_b_last is not None:
            for pi in popped_all:
                add_dep_helper(pi.ins, rec_b_last.ins, sync=False,
                               reason="prefill after this-step rec")
            pending = popped_all
        else:
            pending = pending + popped_all
    for dd in range(2):
        for op in queues[dd]:
            op()

    # ---------------- output projection ----------------
    with tc.tile_pool(name="lps", bufs=1, space="PSUM") as plp:
        lg = plp.tile([128, L2], f32, tag="lg", name="lg")
        for ti in range(NTILE):
            for kk in range(4):
                dd, ch = kk // 2, kk % 2
                nc.tensor.matmul(
                    out=lg[:, ti * 2 : ti * 2 + 2],
                    lhsT=hst[dd][:, ch * T * 8 + ti * 128 :
                                  ch * T * 8 + (ti + 1) * 128],
                    rhs=w_out[:, kk * 2 : kk * 2 + 2],
                    start=(kk == 0), stop=(kk == 3))
        nc.vector.tensor_copy(lg_sb[:], lg[:])
    nc.sync.dma_start(d["logits"].ap(), lg_sb[:])
    gp.release()
    zpool[1].release()
    zpool[0].release()
    pg.release()
    persist.release()


def _build(T=T_FULL):
    if T in _CACHE:
        return _CACHE[T]
    from concourse import bacc, mybir, tile

    f32 = mybir.dt.float32
    bf16 = mybir.dt.bfloat16
    nc = bacc.Bacc("TRN2", target_bir_lowering=False, debug=False,
                   enable_asserts=False, num_devices=NCORE)
    NTOK = BL * T
    NTILE = NTOK // 128
    d = {
        "idx": nc.dram_tensor("idx", [128, NTILE], mybir.dt.int32,
                              kind="ExternalInput"),
        "emb": nc.dram_tensor("emb", [V, E], bf16, kind="ExternalInput"),
        "wx": nc.dram_tensor("wx", [2, 128, 2048], bf16, kind="ExternalInput"),
        "wh": nc.dram_tensor("wh", [2, 128, 2048], bf16, kind="ExternalInput"),
        "bias": nc.dram_tensor("bias", [2, G], bf16, kind="ExternalInput"),
        "wout": nc.dram_tensor("wout", [128, 8], bf16, kind="ExternalInput"),
        "logits": nc.dram_tensor("logits", [128, 2 * NTILE], f32,
                                 kind="ExternalOutput"),
    }
    with tile.TileContext(nc) as tc:
        _emit(nc, tc, d, T)
    nc.compile()
    _CACHE[T] = (nc, d)
    return nc, d


GATE_PERM = np.r_[768:1024, 0:256, 512:768, 256:512]   # [o, i, f, j]


def _stage_core(core, inputs, outputs, mask, emb16, Wk_f, b_f, Wk_b, b_b, W, b, T):
    """Build the per-core input map (pure slicing / transposition / layout).
    emb16 is the embedding table already cast to bf16 (shared across cores)."""
    import ml_dtypes

    k8 = core * BL
    NTOK = BL * T
    NTILE = NTOK // 128
    idx = np.ascontiguousarray(
        inputs[k8 : k8 + BL, :T].T.reshape(NTOK).reshape(NTILE, 128).T
    ).astype(np.int32)
    bf = ml_dtypes.bfloat16
    wx = np.empty((2, 128, 2048), bf)
    wh = np.empty((2, 128, 2048), bf)
    bias = np.empty((2, G), bf)
    for dd, (Wk, bb) in enumerate(((Wk_f, b_f), (Wk_b, b_b))):
        Wp = np.asarray(Wk, np.float32)[:, GATE_PERM].copy()
        bp = np.asarray(bb, np.float32)[GATE_PERM].copy()
        # TF LSTMCell forget bias (permuted order o,i,f,j -> f at 512:768)
        bp[512:768] += 1.0
        # tanh(j) = 2*sigmoid(2j)-1: double the j-gate weights and bias so the
        # one big sigmoid op covers j too (x2 is exact in bf16)
        Wp[:, 768:1024] *= 2.0
        bp[768:1024] *= 2.0
        wx[dd] = (Wp[:E].reshape(2, 128, 8, 128).transpose(1, 0, 2, 3)
                  .reshape(128, 2048).astype(bf))
        wh[dd] = (Wp[E:].reshape(2, 128, 8, 128).transpose(1, 0, 2, 3)
                  .reshape(128, 2048).astype(bf))
        bias[dd] = bp.astype(bf)
    wout = W.reshape(4, 128, 2).transpose(1, 0, 2).reshape(128, 8).astype(bf)
    return {
        "idx": idx,
        "emb": emb16,
        "wx": wx, "wh": wh, "bias": bias,
        "wout": np.ascontiguousarray(wout),
    }


def run(inputs, outputs, mask, emb, Wk_f, b_f, Wk_b, b_b, W, b,
        T=T_FULL, trace=False):
    from concourse import bass_utils

    import ml_dtypes

    nc, d = _build(T)
    emb16 = np.ascontiguousarray(
        np.asarray(emb, np.float32).astype(ml_dtypes.bfloat16))
    args = (np.asarray(inputs), np.asarray(outputs, np.float32),
            np.asarray(mask, np.float32), emb16,
            np.asarray(Wk_f, np.float32), np.asarray(b_f, np.float32),
            np.asarray(Wk_b, np.float32), np.asarray(b_b, np.float32),
            np.asarray(W, np.float32), np.asarray(b, np.float32))
    in_maps = [_stage_core(kc, *args, T) for kc in range(NCORE)]
    res = bass_utils.run_bass_kernel_spmd(nc, in_maps, core_ids=list(range(NCORE)),
                                          trace=trace)
    NTOK = BL * T
    NTILE = NTOK // 128
    # host tail: assemble logits, add b, float64 log-softmax xent, mean
    logits = np.empty((B, T, 2), np.float64)
    for kc in range(NCORE):
        lo = np.asarray(res.results[kc]["logits"], np.float64)   # [128, 2*NTILE]
        lo = lo.reshape(128, NTILE, 2).transpose(1, 0, 2).reshape(NTOK, 2)
        logits[kc * BL : (kc + 1) * BL] = lo.reshape(T, BL, 2).transpose(1, 0, 2)
    logits += np.asarray(b, np.float64)
    m = logits.max(-1, keepdims=True)
    lsm = logits - (m + np.log(np.exp(logits - m).sum(-1, keepdims=True)))
    xent = -(np.asarray(outputs, np.float64)[:, :T] * lsm).sum(-1)
    loss = np.float32((xent * np.asarray(mask, np.float64)[:, :T]).mean())
    return np.asarray(loss), res


def kernel(inputs, outputs, mask, emb, Wk_f, b_f, Wk_b, b_b, W, b):
    loss, _ = run(inputs, outputs, mask, emb, Wk_f, b_f, Wk_b, b_b, W, b)
    return loss


# revision 21
# speedup vs baseline: 1.2232x; 1.0113x over previous
"""Bass/Trainium2 kernel for a BiLSTM (TF-LSTMCell) cross-entropy loss.

Model (per reference):
  x = emb[inputs]                        # [B,T,E]
  h_fw = LSTM(x, Wk_f, b_f)              # forward over T
  h_bw = reverse(LSTM(reverse(x), Wk_b, b_b))
  logits = concat(h_fw, h_bw) @ W + b    # [B,T,2]
  loss = mean(xent(logits, outputs) * mask)

Sharding: data-parallel over batch. B=64 -> 8 cores x 8 rows.  Each core runs
both LSTM directions (two independent chains interleaved to hide latency) and
produces the pre-bias logits for its tokens; the host adds b, computes the
log-softmax cross entropy in float64 and averages (same split as summing the 8
per-core partials: the device does all O(B*T*H) work, the host the O(B*T) tail,
which also avoids an ACT-table swap for LN on device).

Device layout is feature-major: gate/feature index on the SBUF partition axis,
(time x batch) on the free axis, so per-step elementwise work is [128, small]
and the recurrent matmul keeps the weight stationary (bf16 -> fast weight
load).  z pre-activations accumulate in PSUM in 8-step blocks:
  psum col = m*64 + l*8 + b   (l=step-in-block, m=gate chunk of 128, b=batch)
Gate order is host-permuted to [o, i, f, j] so sigmoid covers one contiguous
[128,48] slice and tanh(j) one [128,16].  Weight prep (forget bias +1, j-gate
x2 for the tanh-via-sigmoid trick, bf16 cast, permutation) happens on the host
so weights are DMA-ready.  x-projection and bias are injected into each PSUM
block ahead of time by large-N matmuls (bias via a K=1 ones-row matmul),
keeping the serial chain per step minimal:
  rec-MM (16 bf16 matmuls) -> sigmoid/tanh (ACT) -> cell update (DVE) ->
  tanh(c) (ACT) -> h write (DVE, bf16) -> next rec-MM.

The embedding gather + PE transposes for the input tokens are pipelined into
the first recurrence steps (only the two tiles needed by block 0 are done up
front), so the recurrence starts ~13us into the kernel instead of ~43us.
"""

import numpy as np

B, T_FULL, V, E, H = 64, 256, 32000, 256, 256
G = 4 * H            # 1024 gate dim
NCORE = 8
BL = B // NCORE      # 8 batch rows per core
SB = 8               # recurrence steps per PSUM block

_CACHE = {}


def _emit(nc, tc, d, T):
    """Emit the whole kernel under TileContext tc. d = dict of dram handles."""
    from concourse import bass, mybir

    f32 = mybir.dt.float32
    bf16 = mybir.dt.bfloat16
    AF = mybir.ActivationFunctionType
    OP = mybir.AluOpType

    NTOK = BL * T
    NTILE = NTOK // 128
    NBLK = T // SB
    L2 = 2 * NTILE       # logits tile free dim (classes packed)

    persist = tc.alloc_tile_pool(name="persist", bufs=1)

    # ---------------- persistent SBUF buffers ----------------
    idx_sb = persist.tile([128, NTILE], mybir.dt.int32, tag="idx", name="idx")
    ident = persist.tile([128, 128], bf16, tag="ident", name="ident")
    xT = persist.tile([128, 2 * NTOK], bf16, tag="xT", name="xT")  # [p, k(2), tok]
    wx = [persist.tile([128, 2048], bf16, tag=f"wx{dd}", name=f"wx{dd}") for dd in range(2)]
    wh = [persist.tile([128, 2048], bf16, tag=f"wh{dd}", name=f"wh{dd}") for dd in range(2)]
    bias16 = [persist.tile([1, G], bf16, tag=f"bias16_{dd}", name=f"bias16_{dd}")
              for dd in range(2)]
    hst = [persist.tile([128, 16 * T], bf16, tag=f"h{dd}", name=f"h{dd}") for dd in range(2)]
    ones64 = persist.tile([1, 64], bf16, tag="ones64", name="ones64")
    w_out = persist.tile([128, 8], bf16, tag="w_out", name="w_out")
    lg_sb = persist.tile([128, L2], f32, tag="lg_sb", name="lg_sb")

    # ---------------- constants (gpsimd, ahead of the gathers) -------------
    from concourse.masks import make_identity

    make_identity(nc, ident[:])
    nc.gpsimd.memset(ones64[:], 1.0)

    # ---------------- weight / index loads (sync + scalar queues) ----------
    nc.sync.dma_start(idx_sb[:], d["idx"].ap())
    nc.sync.dma_start(wx[0][:], d["wx"].ap()[0])
    nc.sync.dma_start(wh[0][:], d["wh"].ap()[0])
    nc.scalar.dma_start(wx[1][:], d["wx"].ap()[1])
    nc.scalar.dma_start(wh[1][:], d["wh"].ap()[1])
    for dd in range(2):
        nc.sync.dma_start(bias16[dd][:], d["bias"].ap()[dd : dd + 1])
    nc.scalar.dma_start(w_out[:], d["wout"].ap())

    # ---------------- stage A: gather + xbar transpose ----------------
    # bf16 embedding rows are gathered per 128-token tile (gpsimd SWDGE),
    # then transposed feature-major entirely on the DMA xbar (sync queue) --
    # no PE/ACT/DVE involvement, so stage A never perturbs the recurrence.
    xTr = xT[:].rearrange("p (k n) -> p k n", k=2)
    # interleave from both ends: fw consumes tile 0 first, bw tile NTILE-1
    order = []
    for i in range((NTILE + 1) // 2):
        order.append(i)
        if NTILE - 1 - i > i:
            order.append(NTILE - 1 - i)
    pg = tc.alloc_tile_pool(name="gather", bufs=1)
    xgs = {}

    def gather(i):
        xg = pg.tile([128, E], bf16, tag=f"xg{i}", name=f"xg{i}")
        xgs[i] = xg
        nc.gpsimd.indirect_dma_start(
            out=xg[:], out_offset=None, in_=d["emb"].ap(),
            in_offset=bass.IndirectOffsetOnAxis(ap=idx_sb[:, i : i + 1], axis=0),
        )

    def transpose(i):
        for k in range(2):
            nc.sync.dma_start_transpose(
                out=xTr[:, k, i * 128 : (i + 1) * 128],
                in_=xgs[i][:, k * 128 : (k + 1) * 128])

    # block 0's two tiles gate the first recurrence step: gather them first
    # and transpose on the (idle, cold) PE so they don't queue behind the
    # sem-recycled DMA-transpose stream.  Everything else trails on the
    # gpsimd/sync queues with tens of microseconds of slack.
    for i in order[:2]:
        gather(i)
    with tc.tile_pool(name="tps", bufs=2, space="PSUM") as pps:
        for i in order[:2]:
            for k in range(2):
                ps = pps.tile([128, 128], bf16, tag="tp", name="tp")
                nc.tensor.transpose(out=ps[:], in_=xgs[i][:, k * 128 : (k + 1) * 128],
                                    identity=ident[:])
                nc.vector.tensor_copy(xTr[:, k, i * 128 : (i + 1) * 128], ps[:])
    for i in order[2:]:
        gather(i)
    for i in order[2:]:
        transpose(i)

    # ---------------- recurrence ----------------
    # hst layout: [p, k(2), t(T), b(8)]  (k-major so the loss-stage stationary
    # operand over tokens is a single contiguous free dim)
    hr = [hst[dd][:].rearrange("p (k t b) -> p k t b", k=2, b=8) for dd in range(2)]
    wxr = [wx[dd][:].rearrange("p (q j) -> p q j", j=128) for dd in range(2)]
    whr = [wh[dd][:].rearrange("p (q j) -> p q j", j=128) for dd in range(2)]

    def t0_of(dd, bi):
        return bi * SB if dd == 0 else T - SB - bi * SB

    zpool = [tc.alloc_tile_pool(name=f"z{dd}", bufs=2, space="PSUM")
             for dd in range(2)]
    ztile = [{}, {}]

    def prefill_ops(dd, bi):
        """Closures emitting x-proj + bias matmuls for block bi of dir dd."""
        zt = zpool[dd].tile([128, SB * 64], f32, tag=f"zt{dd}", name=f"zt{dd}")
        ztile[dd][bi] = zt
        # m-major: col = m*64 + l*8 + b -> x-proj/bias matmuls write contiguous
        # column ranges (strided PSUM out-APs measured ~7x slower per matmul)
        ztr = zt[:].rearrange("p (m l b) -> p m l b", l=SB, m=8, b=8)
        t0 = t0_of(dd, bi)
        # local index of global step s=0 in this block (block 0 only);
        # that region gets no recurrent matmul, so bias must close the group
        l_s0 = None
        if bi == 0:
            l_s0 = (0 - t0) if dd == 0 else (T - 1 - t0)
        ops = []
        for m in range(8):
            for k in range(2):
                def op_x(m=m, k=k):
                    return nc.tensor.matmul(
                        out=ztr[:, m, :, :],
                        lhsT=wxr[dd][:, k * 8 + m, :],
                        rhs=xTr[:, k, t0 * 8 : (t0 + SB) * 8],
                        start=(k == 0), stop=False)
                ops.append(op_x)

            def op_b(m=m, l_s0=l_s0):
                if l_s0 is None:
                    return nc.tensor.matmul(out=ztr[:, m, :, :],
                                     lhsT=bias16[dd][:, m * 128 : (m + 1) * 128],
                                     rhs=ones64[:, 0 : SB * 8],
                                     start=False, stop=False)
                else:
                    rest = slice(1, SB) if l_s0 == 0 else slice(0, SB - 1)
                    nc.tensor.matmul(out=ztr[:, m, rest, :],
                                     lhsT=bias16[dd][:, m * 128 : (m + 1) * 128],
                                     rhs=ones64[:, 0 : (SB - 1) * 8],
                                     start=False, stop=False)
                    return nc.tensor.matmul(out=ztr[:, m, l_s0, :],
                                     lhsT=bias16[dd][:, m * 128 : (m + 1) * 128],
                                     rhs=ones64[:, 0:8],
                                     start=False, stop=True)
            ops.append(op_b)
        return ops

    gp = tc.alloc_tile_pool(name="gates", bufs=6)

    # rolling per-step work tiles: cols 0:64 = sigmoid(gates) [o,i,f,j2]
    # written at step s, cols 64:80 = c written by step s-1.  Fresh pool tile
    # per step keeps every write single-assignment (no per-step cross-engine
    # WAR semaphores on a persistent tile).
    cur_w = [None, None]
    for dd in range(2):
        w0 = gp.tile([128, 80], f32, tag=f"wk{dd}", name=f"wk{dd}")
        nc.gpsimd.memset(w0[:, 64:80], 0.0)
        cur_w[dd] = w0

    def step(dd, s):
        bi = s // SB
        t = s if dd == 0 else T - 1 - s
        l = t - t0_of(dd, bi)
        zt = ztile[dd][bi]
        ztr = zt[:].rearrange("p (m l b) -> p m l b", l=SB, m=8, b=8)
        rec_first = rec_last = None
        if s > 0:
            tp = t - 1 if dd == 0 else t + 1
            for m in range(8):
                for k in range(2):
                    mm = nc.tensor.matmul(out=ztr[:, m, l, :],
                                          lhsT=whr[dd][:, k * 8 + m, :],
                                          rhs=hr[dd][:, k, tp, :],
                                          start=False, stop=(k == 1))
                    rec_last = mm
                    if rec_first is None:
                        rec_first = mm
        w = cur_w[dd]
        nxt = gp.tile([128, 80], f32, tag=f"wk{dd}", name=f"wk{dd}")
        cur_w[dd] = nxt
        o = {"rec_first": rec_first, "rec_last": rec_last}
        # one sigmoid over all four gates [o,i,f,j2]; j-weights were doubled
        # so sig_j2 = sigmoid(2j) and tanh(j) = 2*sig_j2 - 1
        o["sig"] = nc.scalar.activation(
            w[:, 0:64].rearrange("p (m b) -> p m b", b=8),
            ztr[:, :, l, :], AF.Sigmoid)
        # paired product: [sig_i*sig_j2 | sig_f*c] in one op
        pm = gp.tile([128, 32], f32, tag="pm", name="pm")
        o["pm"] = nc.vector.tensor_tensor(pm[:], w[:, 16:48], w[:, 48:80],
                                          op=OP.mult)
        # v = 2*sig_i*sig_j2 - sig_i = sig_i * tanh(j)
        vt = gp.tile([128, 16], f32, tag="vt", name="vt")
        o["vt"] = nc.vector.scalar_tensor_tensor(
            out=vt[:], in0=pm[:, 0:16], scalar=2.0, in1=w[:, 16:32],
            op0=OP.mult, op1=OP.subtract)
        # c = sig_f*c + sig_i*tanh(j), written into the NEXT step's work tile
        o["add"] = nc.vector.tensor_tensor(nxt[:, 64:80], vt[:], pm[:, 16:32],
                                           op=OP.add)
        tct = gp.tile([128, 16], f32, tag="tct", name="tct")
        o["tanh"] = nc.scalar.activation(tct[:], nxt[:, 64:80], AF.Tanh)
        o["h"] = nc.vector.tensor_tensor(
            hr[dd][:, :, t, :],
            w[:, 0:16].rearrange("p (k b) -> p k b", k=2),
            tct[:].rearrange("p (k b) -> p k b", k=2),
            op=OP.mult)
        return o

    for op in prefill_ops(0, 0):
        op()
    for op in prefill_ops(1, 0):
        op()
    from concourse.tile_rust import add_dep_helper

    queues = [[], []]
    pending = []
    for s in range(T):
        if s % SB == 0:
            bi = s // SB
            for dd in range(2):
                queues[dd] = prefill_ops(dd, bi + 1) if bi + 1 < NBLK else []
        popped_all = []
        so = [None, None]
        for dd in range(2):
            for _ in range(3):
                if queues[dd]:
                    popped_all.append(queues[dd].pop(0)())
            so[dd] = step(dd, s)
        rec_f_first = so[0]["rec_first"]
        rec_b_last = so[1]["rec_last"]
        # pin the steady-state DVE/ACT interleave (fw offset half a period
        # ahead of bw): pmA vtA addA pmB vtB addB hA hB / sigA sigB tanA tanB.
        # Soft deps only — keeps the scheduler from coupling the two serial
        # chains in an order that stretches the step period.
        oa, ob = so
        add_dep_helper(ob["pm"].ins, oa["add"].ins, sync=False,
                       reason="dve interleave")
        add_dep_helper(oa["tanh"].ins, ob["sig"].ins, sync=False,
                       reason="act interleave")
        add_dep_helper(oa["h"].ins, ob["add"].ins, sync=False,
                       reason="dve interleave")
        add_dep_helper(ob["h"].ins, oa["h"].ins, sync=False,
                       reason="dve interleave")
        # pin prefill into the inter-step PE idle window: after BOTH dirs'
        # recurrent matmuls of this step, before the next step's first
        if rec_f_first is not None:
            for pi in pending:
                add_dep_helper(rec_f_first.ins, pi.ins, sync=False,
                               reason="prefill before next-step rec")
        if rec_b_last is not None:
            for pi in popped_all:
                add_dep_helper(pi.ins, rec_b_last.ins, sync=False,
                               reason="prefill after this-step rec")
            pending = popped_all
        else:
            pending = pending + popped_all
    for dd in range(2):
        for op in queues[dd]:
            op()

    # ---------------- output projection ----------------
    with tc.tile_pool(name="lps", bufs=1, space="PSUM") as plp:
        lg = plp.tile([128, L2], f32, tag="lg", name="lg")
        for ti in range(NTILE):
            for kk in range(4):
                dd, ch = kk // 2, kk % 2
                nc.tensor.matmul(
                    out=lg[:, ti * 2 : ti * 2 + 2],
                    lhsT=hst[dd][:, ch * T * 8 + ti * 128 :
                                  ch * T * 8 + (ti + 1) * 128],
                    rhs=w_out[:, kk * 2 : kk * 2 + 2],
                    start=(kk == 0), stop=(kk == 3))
        nc.vector.tensor_copy(lg_sb[:], lg[:])
    nc.sync.dma_start(d["logits"].ap(), lg_sb[:])
    gp.release()
    zpool[1].release()
    zpool[0].release()
    pg.release()
    persist.release()


def _build(T=T_FULL):
    if T in _CACHE:
        return _CACHE[T]
    from concourse import bacc, mybir, tile

    f32 = mybir.dt.float32
    bf16 = mybir.dt.bfloat16
    nc = bacc.Bacc("TRN2", target_bir_lowering=False, debug=False,
                   enable_asserts=False, num_devices=NCORE)
    NTOK = BL * T
    NTILE = NTOK // 128
    d = {
        "idx": nc.dram_tensor("idx", [128, NTILE], mybir.dt.int32,
                              kind="ExternalInput"),
        "emb": nc.dram_tensor("emb", [V, E], bf16, kind="ExternalInput"),
        "wx": nc.dram_tensor("wx", [2, 128, 2048], bf16, kind="ExternalInput"),
        "wh": nc.dram_tensor("wh", [2, 128, 2048], bf16, kind="ExternalInput"),
        "bias": nc.dram_tensor("bias", [2, G], bf16, kind="ExternalInput"),
        "wout": nc.dram_tensor("wout", [128, 8], bf16, kind="ExternalInput"),
        "logits": nc.dram_tensor("logits", [128, 2 * NTILE], f32,
                                 kind="ExternalOutput"),
    }
    with tile.TileContext(nc) as tc:
        _emit(nc, tc, d, T)
    nc.compile()
    _CACHE[T] = (nc, d)
    return nc, d


GATE_PERM = np.r_[768:1024, 0:256, 512:768, 256:512]   # [o, i, f, j]


def _stage_core(core, inputs, outputs, mask, emb16, Wk_f, b_f, Wk_b, b_b, W, b, T):
    """Build the per-core input map (pure slicing / transposition / layout).
    emb16 is the embedding table already cast to bf16 (shared across cores)."""
    import ml_dtypes

    k8 = core * BL
    NTOK = BL * T
    NTILE = NTOK // 128
    idx = np.ascontiguousarray(
        inputs[k8 : k8 + BL, :T].T.reshape(NTOK).reshape(NTILE, 128).T
    ).astype(np.int32)
    bf = ml_dtypes.bfloat16
    wx = np.empty((2, 128, 2048), bf)
    wh = np.empty((2, 128, 2048), bf)
    bias = np.empty((2, G), bf)
    for dd, (Wk, bb) in enumerate(((Wk_f, b_f), (Wk_b, b_b))):
        Wp = np.asarray(Wk, np.float32)[:, GATE_PERM].copy()
        bp = np.asarray(bb, np.float32)[GATE_PERM].copy()
        # TF LSTMCell forget bias (permuted order o,i,f,j -> f at 512:768)
        bp[512:768] += 1.0
        # tanh(j) = 2*sigmoid(2j)-1: double the j-gate weights and bias so the
        # one big sigmoid op covers j too (x2 is exact in bf16)
        Wp[:, 768:1024] *= 2.0
        bp[768:1024] *= 2.0
        wx[dd] = (Wp[:E].reshape(2, 128, 8, 128).transpose(1, 0, 2, 3)
                  .reshape(128, 2048).astype(bf))
        wh[dd] = (Wp[E:].reshape(2, 128, 8, 128).transpose(1, 0, 2, 3)
                  .reshape(128, 2048).astype(bf))
        bias[dd] = bp.astype(bf)
    wout = W.reshape(4, 128, 2).transpose(1, 0, 2).reshape(128, 8).astype(bf)
    return {
        "idx": idx,
        "emb": emb16,
        "wx": wx, "wh": wh, "bias": bias,
        "wout": np.ascontiguousarray(wout),
    }


def run(inputs, outputs, mask, emb, Wk_f, b_f, Wk_b, b_b, W, b,
        T=T_FULL, trace=False):
    from concourse import bass_utils

    import ml_dtypes

    nc, d = _build(T)
    emb16 = np.ascontiguousarray(
        np.asarray(emb, np.float32).astype(ml_dtypes.bfloat16))
    args = (np.asarray(inputs), np.asarray(outputs, np.float32),
            np.asarray(mask, np.float32), emb16,
            np.asarray(Wk_f, np.float32), np.asarray(b_f, np.float32),
            np.asarray(Wk_b, np.float32), np.asarray(b_b, np.float32),
            np.asarray(W, np.float32), np.asarray(b, np.float32))
    in_maps = [_stage_core(kc, *args, T) for kc in range(NCORE)]
    res = bass_utils.run_bass_kernel_spmd(nc, in_maps, core_ids=list(range(NCORE)),
                                          trace=trace)
    NTOK = BL * T
    NTILE = NTOK // 128
    # host tail: assemble logits, add b, float64 log-softmax xent, mean
    logits = np.empty((B, T, 2), np.float64)
    for kc in range(NCORE):
        lo = np.asarray(res.results[kc]["logits"], np.float64)   # [128, 2*NTILE]
        lo = lo.reshape(128, NTILE, 2).transpose(1, 0, 2).reshape(NTOK, 2)
        logits[kc * BL : (kc + 1) * BL] = lo.reshape(T, BL, 2).transpose(1, 0, 2)
    logits += np.asarray(b, np.float64)
    m = logits.max(-1, keepdims=True)
    lsm = logits - (m + np.log(np.exp(logits - m).sum(-1, keepdims=True)))
    xent = -(np.asarray(outputs, np.float64)[:, :T] * lsm).sum(-1)
    loss = np.float32((xent * np.asarray(mask, np.float64)[:, :T]).mean())
    return np.asarray(loss), res


def kernel(inputs, outputs, mask, emb, Wk_f, b_f, Wk_b, b_b, W, b):
    loss, _ = run(inputs, outputs, mask, emb, Wk_f, b_f, Wk_b, b_b, W, b)
    return loss


# revision 23
# speedup vs baseline: 1.2500x; 1.0219x over previous
"""Bass/Trainium2 kernel for a BiLSTM (TF-LSTMCell) cross-entropy loss.

Model (per reference):
  x = emb[inputs]                        # [B,T,E]
  h_fw = LSTM(x, Wk_f, b_f)              # forward over T
  h_bw = reverse(LSTM(reverse(x), Wk_b, b_b))
  logits = concat(h_fw, h_bw) @ W + b    # [B,T,2]
  loss = mean(xent(logits, outputs) * mask)

Sharding: data-parallel over batch. B=64 -> 8 cores x 8 rows.  Each core runs
both LSTM directions (two independent chains interleaved to hide latency) and
produces the pre-bias logits for its tokens; the host adds b, computes the
log-softmax cross entropy in float64 and averages (same split as summing the 8
per-core partials: the device does all O(B*T*H) work, the host the O(B*T) tail,
which also avoids an ACT-table swap for LN on device).

Device layout is feature-major: gate/feature index on the SBUF partition axis,
(time x batch) on the free axis, so per-step elementwise work is [128, small]
and the recurrent matmul keeps the weight stationary (bf16 -> fast weight
load).  z pre-activations accumulate in PSUM in 8-step blocks:
  psum col = m*64 + l*8 + b   (l=step-in-block, m=gate chunk of 128, b=batch)
Gate order is host-permuted to [o, i, f, j] so sigmoid covers one contiguous
[128,48] slice and tanh(j) one [128,16].  Weight prep (forget bias +1, j-gate
x2 for the tanh-via-sigmoid trick, bf16 cast, permutation) happens on the host
so weights are DMA-ready.  x-projection and bias are injected into each PSUM
block ahead of time by large-N matmuls (bias via a K=1 ones-row matmul),
keeping the serial chain per step minimal:
  rec-MM (16 bf16 matmuls) -> sigmoid/tanh (ACT) -> cell update (DVE) ->
  tanh(c) (ACT) -> h write (DVE, bf16) -> next rec-MM.

The embedding gather + PE transposes for the input tokens are pipelined into
the first recurrence steps (only the two tiles needed by block 0 are done up
front), so the recurrence starts ~13us into the kernel instead of ~43us.
"""

import numpy as np

B, T_FULL, V, E, H = 64, 256, 32000, 256, 256
G = 4 * H            # 1024 gate dim
NCORE = 8
BL = B // NCORE      # 8 batch rows per core
SB = 8               # recurrence steps per PSUM block

_CACHE = {}


def _emit(nc, tc, d, T):
    """Emit the whole kernel under TileContext tc. d = dict of dram handles."""
    from concourse import bass, mybir

    f32 = mybir.dt.float32
    bf16 = mybir.dt.bfloat16
    AF = mybir.ActivationFunctionType
    OP = mybir.AluOpType

    NTOK = BL * T
    NTILE = NTOK // 128
    NBLK = T // SB
    L2 = 2 * NTILE       # logits tile free dim (classes packed)

    persist = tc.alloc_tile_pool(name="persist", bufs=1)

    # ---------------- persistent SBUF buffers ----------------
    idx_sb = persist.tile([128, NTILE], mybir.dt.int32, tag="idx", name="idx")
    ident = persist.tile([128, 128], bf16, tag="ident", name="ident")
    xT = persist.tile([128, 2 * NTOK], bf16, tag="xT", name="xT")  # [p, k(2), tok]
    wx = [persist.tile([128, 2048], bf16, tag=f"wx{dd}", name=f"wx{dd}") for dd in range(2)]
    wh = [persist.tile([128, 2048], bf16, tag=f"wh{dd}", name=f"wh{dd}") for dd in range(2)]
    bias16 = [persist.tile([1, G], bf16, tag=f"bias16_{dd}", name=f"bias16_{dd}")
              for dd in range(2)]
    hst = [persist.tile([128, 16 * T], bf16, tag=f"h{dd}", name=f"h{dd}") for dd in range(2)]
    ones64 = persist.tile([1, 64], bf16, tag="ones64", name="ones64")
    w_out = persist.tile([128, 8], bf16, tag="w_out", name="w_out")
    lg_sb = persist.tile([128, L2], f32, tag="lg_sb", name="lg_sb")

    # ---------------- constants (gpsimd, ahead of the gathers) -------------
    from concourse.masks import make_identity

    make_identity(nc, ident[:])
    nc.gpsimd.memset(ones64[:], 1.0)

    # ---------------- weight / index loads (sync + scalar queues) ----------
    nc.sync.dma_start(idx_sb[:], d["idx"].ap())
    nc.sync.dma_start(wx[0][:], d["wx"].ap()[0])
    nc.sync.dma_start(wh[0][:], d["wh"].ap()[0])
    nc.scalar.dma_start(wx[1][:], d["wx"].ap()[1])
    nc.scalar.dma_start(wh[1][:], d["wh"].ap()[1])
    for dd in range(2):
        nc.sync.dma_start(bias16[dd][:], d["bias"].ap()[dd : dd + 1])
    nc.scalar.dma_start(w_out[:], d["wout"].ap())

    # ---------------- stage A: gather + xbar transpose ----------------
    # bf16 embedding rows are gathered per 128-token tile (gpsimd SWDGE),
    # then transposed feature-major entirely on the DMA xbar (sync queue) --
    # no PE/ACT/DVE involvement, so stage A never perturbs the recurrence.
    xTr = xT[:].rearrange("p (k n) -> p k n", k=2)
    # interleave from both ends: fw consumes tile 0 first, bw tile NTILE-1
    order = []
    for i in range((NTILE + 1) // 2):
        order.append(i)
        if NTILE - 1 - i > i:
            order.append(NTILE - 1 - i)
    pg = tc.alloc_tile_pool(name="gather", bufs=1)
    xgs = {}

    def gather(i):
        xg = pg.tile([128, E], bf16, tag=f"xg{i}", name=f"xg{i}")
        xgs[i] = xg
        nc.gpsimd.indirect_dma_start(
            out=xg[:], out_offset=None, in_=d["emb"].ap(),
            in_offset=bass.IndirectOffsetOnAxis(ap=idx_sb[:, i : i + 1], axis=0),
        )

    def transpose(i):
        for k in range(2):
            nc.sync.dma_start_transpose(
                out=xTr[:, k, i * 128 : (i + 1) * 128],
                in_=xgs[i][:, k * 128 : (k + 1) * 128])

    # block 0's two tiles gate the first recurrence step: gather them first
    # and transpose on the (idle, cold) PE so they don't queue behind the
    # sem-recycled DMA-transpose stream.  Everything else trails on the
    # gpsimd/sync queues with tens of microseconds of slack.
    for i in order[:2]:
        gather(i)
    with tc.tile_pool(name="tps", bufs=2, space="PSUM") as pps:
        for i in order[:2]:
            for k in range(2):
                ps = pps.tile([128, 128], bf16, tag="tp", name="tp")
                nc.tensor.transpose(out=ps[:], in_=xgs[i][:, k * 128 : (k + 1) * 128],
                                    identity=ident[:])
                nc.vector.tensor_copy(xTr[:, k, i * 128 : (i + 1) * 128], ps[:])
    for i in order[2:]:
        gather(i)
    for i in order[2:]:
        transpose(i)

    # ---------------- recurrence ----------------
    # hst layout: [p, k(2), t(T), b(8)]  (k-major so the loss-stage stationary
    # operand over tokens is a single contiguous free dim)
    hr = [hst[dd][:].rearrange("p (k t b) -> p k t b", k=2, b=8) for dd in range(2)]
    wxr = [wx[dd][:].rearrange("p (q j) -> p q j", j=128) for dd in range(2)]
    whr = [wh[dd][:].rearrange("p (q j) -> p q j", j=128) for dd in range(2)]

    def t0_of(dd, bi):
        return bi * SB if dd == 0 else T - SB - bi * SB

    zpool = [tc.alloc_tile_pool(name=f"z{dd}", bufs=2, space="PSUM")
             for dd in range(2)]
    ztile = [{}, {}]

    def prefill_ops(dd, bi):
        """Closures emitting x-proj + bias matmuls for block bi of dir dd."""
        zt = zpool[dd].tile([128, SB * 64], f32, tag=f"zt{dd}", name=f"zt{dd}")
        ztile[dd][bi] = zt
        # m-major: col = m*64 + l*8 + b -> x-proj/bias matmuls write contiguous
        # column ranges (strided PSUM out-APs measured ~7x slower per matmul)
        ztr = zt[:].rearrange("p (m l b) -> p m l b", l=SB, m=8, b=8)
        t0 = t0_of(dd, bi)
        # local index of global step s=0 in this block (block 0 only);
        # that region gets no recurrent matmul, so bias must close the group
        l_s0 = None
        if bi == 0:
            l_s0 = (0 - t0) if dd == 0 else (T - 1 - t0)
        ops = []
        for m in range(8):
            for k in range(2):
                def op_x(m=m, k=k):
                    return nc.tensor.matmul(
                        out=ztr[:, m, :, :],
                        lhsT=wxr[dd][:, k * 8 + m, :],
                        rhs=xTr[:, k, t0 * 8 : (t0 + SB) * 8],
                        start=(k == 0), stop=False)
                ops.append(op_x)

            def op_b(m=m, l_s0=l_s0):
                if l_s0 is None:
                    return nc.tensor.matmul(out=ztr[:, m, :, :],
                                     lhsT=bias16[dd][:, m * 128 : (m + 1) * 128],
                                     rhs=ones64[:, 0 : SB * 8],
                                     start=False, stop=False)
                else:
                    rest = slice(1, SB) if l_s0 == 0 else slice(0, SB - 1)
                    nc.tensor.matmul(out=ztr[:, m, rest, :],
                                     lhsT=bias16[dd][:, m * 128 : (m + 1) * 128],
                                     rhs=ones64[:, 0 : (SB - 1) * 8],
                                     start=False, stop=False)
                    return nc.tensor.matmul(out=ztr[:, m, l_s0, :],
                                     lhsT=bias16[dd][:, m * 128 : (m + 1) * 128],
                                     rhs=ones64[:, 0:8],
                                     start=False, stop=True)
            ops.append(op_b)
        return ops

    gp = tc.alloc_tile_pool(name="gates", bufs=6)

    # rolling per-step work tiles: cols 0:64 = sigmoid(gates) [o,i,f,j2]
    # written at step s, cols 64:80 = c written by step s-1.  Fresh pool tile
    # per step keeps every write single-assignment (no per-step cross-engine
    # WAR semaphores on a persistent tile).
    cur_w = [None, None]
    for dd in range(2):
        w0 = gp.tile([128, 80], f32, tag=f"wk{dd}", name=f"wk{dd}")
        nc.gpsimd.memset(w0[:, 64:80], 0.0)
        cur_w[dd] = w0

    def step(dd, s):
        bi = s // SB
        t = s if dd == 0 else T - 1 - s
        l = t - t0_of(dd, bi)
        zt = ztile[dd][bi]
        ztr = zt[:].rearrange("p (m l b) -> p m l b", l=SB, m=8, b=8)
        rec_first = rec_last = None
        if s > 0:
            tp = t - 1 if dd == 0 else t + 1
            # o-gate chunks (m=0,1) issue LAST so the [i,f,j] sigmoid can
            # start after only 12 of the 16 matmuls — the o sigmoid runs as a
            # separate ACT op off the critical chain (h needs it only after
            # tanh(c), by which time it is long done)
            for m in (2, 3, 4, 5, 6, 7, 0, 1):
                for k in range(2):
                    mm = nc.tensor.matmul(out=ztr[:, m, l, :],
                                          lhsT=whr[dd][:, k * 8 + m, :],
                                          rhs=hr[dd][:, k, tp, :],
                                          start=False, stop=(k == 1))
                    rec_last = mm
                    if rec_first is None:
                        rec_first = mm
        w = cur_w[dd]
        nxt = gp.tile([128, 80], f32, tag=f"wk{dd}", name=f"wk{dd}")
        cur_w[dd] = nxt
        o = {"rec_first": rec_first, "rec_last": rec_last}
        # sigmoid over [i,f,j2] (j-weights doubled so sig_j2 = sigmoid(2j)
        # and tanh(j) = 2*sig_j2 - 1); o-gates in a separate ACT op
        o["sig"] = nc.scalar.activation(
            w[:, 16:64].rearrange("p (m b) -> p m b", b=8),
            ztr[:, 2:8, l, :], AF.Sigmoid)
        o["sigo"] = nc.scalar.activation(
            w[:, 0:16].rearrange("p (m b) -> p m b", b=8),
            ztr[:, 0:2, l, :], AF.Sigmoid)
        # paired product: [sig_i*sig_j2 | sig_f*c] in one op
        pm = gp.tile([128, 32], f32, tag="pm", name="pm")
        o["pm"] = nc.vector.tensor_tensor(pm[:], w[:, 16:48], w[:, 48:80],
                                          op=OP.mult)
        # v = 2*sig_i*sig_j2 - sig_i = sig_i * tanh(j)
        vt = gp.tile([128, 16], f32, tag="vt", name="vt")
        o["vt"] = nc.vector.scalar_tensor_tensor(
            out=vt[:], in0=pm[:, 0:16], scalar=2.0, in1=w[:, 16:32],
            op0=OP.mult, op1=OP.subtract)
        # c = sig_f*c + sig_i*tanh(j), written into the NEXT step's work tile
        o["add"] = nc.vector.tensor_tensor(nxt[:, 64:80], vt[:], pm[:, 16:32],
                                           op=OP.add)
        tct = gp.tile([128, 16], f32, tag="tct", name="tct")
        o["tanh"] = nc.scalar.activation(tct[:], nxt[:, 64:80], AF.Tanh)
        o["h"] = nc.vector.tensor_tensor(
            hr[dd][:, :, t, :],
            w[:, 0:16].rearrange("p (k b) -> p k b", k=2),
            tct[:].rearrange("p (k b) -> p k b", k=2),
            op=OP.mult)
        return o

    for op in prefill_ops(0, 0):
        op()
    for op in prefill_ops(1, 0):
        op()
    from concourse.tile_rust import add_dep_helper

    queues = [[], []]
    pending = []
    for s in range(T):
        if s % SB == 0:
            bi = s // SB
            for dd in range(2):
                queues[dd] = prefill_ops(dd, bi + 1) if bi + 1 < NBLK else []
        popped_all = []
        so = [None, None]
        for dd in range(2):
            for _ in range(3):
                if queues[dd]:
                    popped_all.append(queues[dd].pop(0)())
            so[dd] = step(dd, s)
        rec_f_first = so[0]["rec_first"]
        rec_b_last = so[1]["rec_last"]
        # pin the steady-state DVE/ACT interleave (fw offset half a period
        # ahead of bw): pmA vtA addA pmB vtB addB hA hB / sigA sigB tanA tanB.
        # Soft deps only — keeps the scheduler from coupling the two serial
        # chains in an order that stretches the step period.
        oa, ob = so
        add_dep_helper(ob["pm"].ins, oa["add"].ins, sync=False,
                       reason="dve interleave")
        add_dep_helper(oa["sigo"].ins, oa["sig"].ins, sync=False,
                       reason="act interleave")
        add_dep_helper(ob["sig"].ins, oa["sigo"].ins, sync=False,
                       reason="act interleave")
        add_dep_helper(ob["sigo"].ins, ob["sig"].ins, sync=False,
                       reason="act interleave")
        add_dep_helper(oa["tanh"].ins, ob["sigo"].ins, sync=False,
                       reason="act interleave")
        add_dep_helper(oa["h"].ins, ob["add"].ins, sync=False,
                       reason="dve interleave")
        add_dep_helper(ob["h"].ins, oa["h"].ins, sync=False,
                       reason="dve interleave")
        # pin prefill into the inter-step PE idle window: after BOTH dirs'
        # recurrent matmuls of this step, before the next step's first
        if rec_f_first is not None:
            for pi in pending:
                add_dep_helper(rec_f_first.ins, pi.ins, sync=False,
                               reason="prefill before next-step rec")
        if rec_b_last is not None:
            for pi in popped_all:
                add_dep_helper(pi.ins, rec_b_last.ins, sync=False,
                               reason="prefill after this-step rec")
            pending = popped_all
        else:
            pending = pending + popped_all
    for dd in range(2):
        for op in queues[dd]:
            op()

    # ---------------- output projection ----------------
    with tc.tile_pool(name="lps", bufs=1, space="PSUM") as plp:
        lg = plp.tile([128, L2], f32, tag="lg", name="lg")
        for ti in range(NTILE):
            for kk in range(4):
                dd, ch = kk // 2, kk % 2
                nc.tensor.matmul(
                    out=lg[:, ti * 2 : ti * 2 + 2],
                    lhsT=hst[dd][:, ch * T * 8 + ti * 128 :
                                  ch * T * 8 + (ti + 1) * 128],
                    rhs=w_out[:, kk * 2 : kk * 2 + 2],
                    start=(kk == 0), stop=(kk == 3))
        nc.vector.tensor_copy(lg_sb[:], lg[:])
    nc.sync.dma_start(d["logits"].ap(), lg_sb[:])
    gp.release()
    zpool[1].release()
    zpool[0].release()
    pg.release()
    persist.release()


def _build(T=T_FULL):
    if T in _CACHE:
        return _CACHE[T]
    from concourse import bacc, mybir, tile

    f32 = mybir.dt.float32
    bf16 = mybir.dt.bfloat16
    nc = bacc.Bacc("TRN2", target_bir_lowering=False, debug=False,
                   enable_asserts=False, num_devices=NCORE)
    NTOK = BL * T
    NTILE = NTOK // 128
    d = {
        "idx": nc.dram_tensor("idx", [128, NTILE], mybir.dt.int32,
                              kind="ExternalInput"),
        "emb": nc.dram_tensor("emb", [V, E], bf16, kind="ExternalInput"),
        "wx": nc.dram_tensor("wx", [2, 128, 2048], bf16, kind="ExternalInput"),
        "wh": nc.dram_tensor("wh", [2, 128, 2048], bf16, kind="ExternalInput"),
        "bias": nc.dram_tensor("bias", [2, G], bf16, kind="ExternalInput"),
        "wout": nc.dram_tensor("wout", [128, 8], bf16, kind="ExternalInput"),
        "logits": nc.dram_tensor("logits", [128, 2 * NTILE], f32,
                                 kind="ExternalOutput"),
    }
    with tile.TileContext(nc) as tc:
        _emit(nc, tc, d, T)
    nc.compile()
    _CACHE[T] = (nc, d)
    return nc, d


GATE_PERM = np.r_[768:1024, 0:256, 512:768, 256:512]   # [o, i, f, j]


def _stage_core(core, inputs, outputs, mask, emb16, Wk_f, b_f, Wk_b, b_b, W, b, T):
    """Build the per-core input map (pure slicing / transposition / layout).
    emb16 is the embedding table already cast to bf16 (shared across cores)."""
    import ml_dtypes

    k8 = core * BL
    NTOK = BL * T
    NTILE = NTOK // 128
    idx = np.ascontiguousarray(
        inputs[k8 : k8 + BL, :T].T.reshape(NTOK).reshape(NTILE, 128).T
    ).astype(np.int32)
    bf = ml_dtypes.bfloat16
    wx = np.empty((2, 128, 2048), bf)
    wh = np.empty((2, 128, 2048), bf)
    bias = np.empty((2, G), bf)
    for dd, (Wk, bb) in enumerate(((Wk_f, b_f), (Wk_b, b_b))):
        Wp = np.asarray(Wk, np.float32)[:, GATE_PERM].copy()
        bp = np.asarray(bb, np.float32)[GATE_PERM].copy()
        # TF LSTMCell forget bias (permuted order o,i,f,j -> f at 512:768)
        bp[512:768] += 1.0
        # tanh(j) = 2*sigmoid(2j)-1: double the j-gate weights and bias so the
        # one big sigmoid op covers j too (x2 is exact in bf16)
        Wp[:, 768:1024] *= 2.0
        bp[768:1024] *= 2.0
        wx[dd] = (Wp[:E].reshape(2, 128, 8, 128).transpose(1, 0, 2, 3)
                  .reshape(128, 2048).astype(bf))
        wh[dd] = (Wp[E:].reshape(2, 128, 8, 128).transpose(1, 0, 2, 3)
                  .reshape(128, 2048).astype(bf))
        bias[dd] = bp.astype(bf)
    wout = W.reshape(4, 128, 2).transpose(1, 0, 2).reshape(128, 8).astype(bf)
    return {
        "idx": idx,
        "emb": emb16,
        "wx": wx, "wh": wh, "bias": bias,
        "wout": np.ascontiguousarray(wout),
    }


def run(inputs, outputs, mask, emb, Wk_f, b_f, Wk_b, b_b, W, b,
        T=T_FULL, trace=False):
    from concourse import bass_utils

    import ml_dtypes

    nc, d = _build(T)
    emb16 = np.ascontiguousarray(
        np.asarray(emb, np.float32).astype(ml_dtypes.bfloat16))
    args = (np.asarray(inputs), np.asarray(outputs, np.float32),
            np.asarray(mask, np.float32), emb16,
            np.asarray(Wk_f, np.float32), np.asarray(b_f, np.float32),
            np.asarray(Wk_b, np.float32), np.asarray(b_b, np.float32),
            np.asarray(W, np.float32), np.asarray(b, np.float32))
    in_maps = [_stage_core(kc, *args, T) for kc in range(NCORE)]
    res = bass_utils.run_bass_kernel_spmd(nc, in_maps, core_ids=list(range(NCORE)),
                                          trace=trace)
    NTOK = BL * T
    NTILE = NTOK // 128
    # host tail: assemble logits, add b, float64 log-softmax xent, mean
    logits = np.empty((B, T, 2), np.float64)
    for kc in range(NCORE):
        lo = np.asarray(res.results[kc]["logits"], np.float64)   # [128, 2*NTILE]
        lo = lo.reshape(128, NTILE, 2).transpose(1, 0, 2).reshape(NTOK, 2)
        logits[kc * BL : (kc + 1) * BL] = lo.reshape(T, BL, 2).transpose(1, 0, 2)
    logits += np.asarray(b, np.float64)
    m = logits.max(-1, keepdims=True)
    lsm = logits - (m + np.log(np.exp(logits - m).sum(-1, keepdims=True)))
    xent = -(np.asarray(outputs, np.float64)[:, :T] * lsm).sum(-1)
    loss = np.float32((xent * np.asarray(mask, np.float64)[:, :T]).mean())
    return np.asarray(loss), res


def kernel(inputs, outputs, mask, emb, Wk_f, b_f, Wk_b, b_b, W, b):
    loss, _ = run(inputs, outputs, mask, emb, Wk_f, b_f, Wk_b, b_b, W, b)
    return loss


# revision 26
# speedup vs baseline: 1.2534x; 1.0027x over previous
"""Bass/Trainium2 kernel for a BiLSTM (TF-LSTMCell) cross-entropy loss.

Model (per reference):
  x = emb[inputs]                        # [B,T,E]
  h_fw = LSTM(x, Wk_f, b_f)              # forward over T
  h_bw = reverse(LSTM(reverse(x), Wk_b, b_b))
  logits = concat(h_fw, h_bw) @ W + b    # [B,T,2]
  loss = mean(xent(logits, outputs) * mask)

Sharding: data-parallel over batch. B=64 -> 8 cores x 8 rows.  Each core runs
both LSTM directions (two independent chains interleaved to hide latency) and
produces the pre-bias logits for its tokens; the host adds b, computes the
log-softmax cross entropy in float64 and averages (same split as summing the 8
per-core partials: the device does all O(B*T*H) work, the host the O(B*T) tail,
which also avoids an ACT-table swap for LN on device).

Device layout is feature-major: gate/feature index on the SBUF partition axis,
(time x batch) on the free axis, so per-step elementwise work is [128, small]
and the recurrent matmul keeps the weight stationary (bf16 -> fast weight
load).  z pre-activations accumulate in PSUM in 8-step blocks:
  psum col = m*64 + l*8 + b   (l=step-in-block, m=gate chunk of 128, b=batch)
Gate order is host-permuted to [o, i, f, j] so sigmoid covers one contiguous
[128,48] slice and tanh(j) one [128,16].  Weight prep (forget bias +1, j-gate
x2 for the tanh-via-sigmoid trick, bf16 cast, permutation) happens on the host
so weights are DMA-ready.  x-projection and bias are injected into each PSUM
block ahead of time by large-N matmuls (bias via a K=1 ones-row matmul),
keeping the serial chain per step minimal:
  rec-MM (16 bf16 matmuls) -> sigmoid/tanh (ACT) -> cell update (DVE) ->
  tanh(c) (ACT) -> h write (DVE, bf16) -> next rec-MM.

The embedding gather + PE transposes for the input tokens are pipelined into
the first recurrence steps (only the two tiles needed by block 0 are done up
front), so the recurrence starts ~13us into the kernel instead of ~43us.
"""

import numpy as np

B, T_FULL, V, E, H = 64, 256, 32000, 256, 256
G = 4 * H            # 1024 gate dim
NCORE = 8
BL = B // NCORE      # 8 batch rows per core
SB = 8               # recurrence steps per PSUM block

_CACHE = {}


def _emit(nc, tc, d, T):
    """Emit the whole kernel under TileContext tc. d = dict of dram handles."""
    from concourse import bass, mybir

    f32 = mybir.dt.float32
    bf16 = mybir.dt.bfloat16
    AF = mybir.ActivationFunctionType
    OP = mybir.AluOpType

    NTOK = BL * T
    NTILE = NTOK // 128
    NBLK = T // SB
    L2 = 2 * NTILE       # logits tile free dim (classes packed)

    persist = tc.alloc_tile_pool(name="persist", bufs=1)

    # ---------------- persistent SBUF buffers ----------------
    idx_sb = persist.tile([128, NTILE], mybir.dt.int32, tag="idx", name="idx")
    ident = persist.tile([128, 128], bf16, tag="ident", name="ident")
    xT = persist.tile([128, 2 * NTOK], bf16, tag="xT", name="xT")  # [p, k(2), tok]
    wx = [persist.tile([128, 2048], bf16, tag=f"wx{dd}", name=f"wx{dd}") for dd in range(2)]
    wh = [persist.tile([128, 2048], bf16, tag=f"wh{dd}", name=f"wh{dd}") for dd in range(2)]
    bias16 = [persist.tile([1, G], bf16, tag=f"bias16_{dd}", name=f"bias16_{dd}")
              for dd in range(2)]
    hst = [persist.tile([128, 16 * T], bf16, tag=f"h{dd}", name=f"h{dd}") for dd in range(2)]
    ones64 = persist.tile([1, 64], bf16, tag="ones64", name="ones64")
    w_out = persist.tile([128, 8], bf16, tag="w_out", name="w_out")
    lg_sb = persist.tile([128, L2], f32, tag="lg_sb", name="lg_sb")

    # ---------------- constants (gpsimd, ahead of the gathers) -------------
    from concourse.masks import make_identity

    make_identity(nc, ident[:])
    nc.gpsimd.memset(ones64[:], 1.0)

    # ---------------- weight / index loads (sync + scalar queues) ----------
    nc.sync.dma_start(idx_sb[:], d["idx"].ap())
    nc.sync.dma_start(wx[0][:], d["wx"].ap()[0])
    nc.sync.dma_start(wh[0][:], d["wh"].ap()[0])
    nc.scalar.dma_start(wx[1][:], d["wx"].ap()[1])
    nc.scalar.dma_start(wh[1][:], d["wh"].ap()[1])
    for dd in range(2):
        nc.sync.dma_start(bias16[dd][:], d["bias"].ap()[dd : dd + 1])
    nc.scalar.dma_start(w_out[:], d["wout"].ap())

    # ---------------- stage A: gather + xbar transpose ----------------
    # bf16 embedding rows are gathered per 128-token tile (gpsimd SWDGE),
    # then transposed feature-major entirely on the DMA xbar (sync queue) --
    # no PE/ACT/DVE involvement, so stage A never perturbs the recurrence.
    xTr = xT[:].rearrange("p (k n) -> p k n", k=2)
    # interleave from both ends: fw consumes tile 0 first, bw tile NTILE-1
    order = []
    for i in range((NTILE + 1) // 2):
        order.append(i)
        if NTILE - 1 - i > i:
            order.append(NTILE - 1 - i)
    pg = tc.alloc_tile_pool(name="gather", bufs=1)
    xgs = {}

    def gather(i):
        xg = pg.tile([128, E], bf16, tag=f"xg{i}", name=f"xg{i}")
        xgs[i] = xg
        nc.gpsimd.indirect_dma_start(
            out=xg[:], out_offset=None, in_=d["emb"].ap(),
            in_offset=bass.IndirectOffsetOnAxis(ap=idx_sb[:, i : i + 1], axis=0),
        )

    def transpose(i):
        for k in range(2):
            nc.sync.dma_start_transpose(
                out=xTr[:, k, i * 128 : (i + 1) * 128],
                in_=xgs[i][:, k * 128 : (k + 1) * 128])

    # block 0's two tiles gate the first recurrence step: gather them first
    # and transpose on the (idle, cold) PE so they don't queue behind the
    # sem-recycled DMA-transpose stream.  Everything else trails on the
    # gpsimd/sync queues with tens of microseconds of slack.
    for i in order[:2]:
        gather(i)
    with tc.tile_pool(name="tps", bufs=2, space="PSUM") as pps:
        for i in order[:2]:
            for k in range(2):
                ps = pps.tile([128, 128], bf16, tag="tp", name="tp")
                nc.tensor.transpose(out=ps[:], in_=xgs[i][:, k * 128 : (k + 1) * 128],
                                    identity=ident[:])
                nc.vector.tensor_copy(xTr[:, k, i * 128 : (i + 1) * 128], ps[:])
    for i in order[2:]:
        gather(i)
    for i in order[2:]:
        transpose(i)

    # ---------------- recurrence ----------------
    # hst layout: [p, k(2), t(T), b(8)]  (k-major so the loss-stage stationary
    # operand over tokens is a single contiguous free dim)
    hr = [hst[dd][:].rearrange("p (k t b) -> p k t b", k=2, b=8) for dd in range(2)]
    wxr = [wx[dd][:].rearrange("p (q j) -> p q j", j=128) for dd in range(2)]
    whr = [wh[dd][:].rearrange("p (q j) -> p q j", j=128) for dd in range(2)]

    def t0_of(dd, bi):
        return bi * SB if dd == 0 else T - SB - bi * SB

    zpool = [tc.alloc_tile_pool(name=f"z{dd}", bufs=2, space="PSUM")
             for dd in range(2)]
    ztile = [{}, {}]

    def prefill_ops(dd, bi):
        """Closures emitting x-proj + bias matmuls for block bi of dir dd."""
        zt = zpool[dd].tile([128, SB * 64], f32, tag=f"zt{dd}", name=f"zt{dd}")
        ztile[dd][bi] = zt
        # m-major: col = m*64 + l*8 + b -> x-proj/bias matmuls write contiguous
        # column ranges (strided PSUM out-APs measured ~7x slower per matmul)
        ztr = zt[:].rearrange("p (m l b) -> p m l b", l=SB, m=8, b=8)
        t0 = t0_of(dd, bi)
        # local index of global step s=0 in this block (block 0 only);
        # that region gets no recurrent matmul, so bias must close the group
        l_s0 = None
        if bi == 0:
            l_s0 = (0 - t0) if dd == 0 else (T - 1 - t0)
        ops = []
        for m in range(8):
            for k in range(2):
                def op_x(m=m, k=k):
                    return nc.tensor.matmul(
                        out=ztr[:, m, :, :],
                        lhsT=wxr[dd][:, k * 8 + m, :],
                        rhs=xTr[:, k, t0 * 8 : (t0 + SB) * 8],
                        start=(k == 0), stop=False)
                ops.append(op_x)

            def op_b(m=m, l_s0=l_s0):
                if l_s0 is None:
                    return nc.tensor.matmul(out=ztr[:, m, :, :],
                                     lhsT=bias16[dd][:, m * 128 : (m + 1) * 128],
                                     rhs=ones64[:, 0 : SB * 8],
                                     start=False, stop=False)
                else:
                    rest = slice(1, SB) if l_s0 == 0 else slice(0, SB - 1)
                    nc.tensor.matmul(out=ztr[:, m, rest, :],
                                     lhsT=bias16[dd][:, m * 128 : (m + 1) * 128],
                                     rhs=ones64[:, 0 : (SB - 1) * 8],
                                     start=False, stop=False)
                    return nc.tensor.matmul(out=ztr[:, m, l_s0, :],
                                     lhsT=bias16[dd][:, m * 128 : (m + 1) * 128],
                                     rhs=ones64[:, 0:8],
                                     start=False, stop=True)
            ops.append(op_b)
        return ops

    gp = tc.alloc_tile_pool(name="gates", bufs=6)

    # rolling per-step work tiles: cols 0:64 = sigmoid(gates) [o,i,f,j2]
    # written at step s, cols 64:80 = c written by step s-1.  Fresh pool tile
    # per step keeps every write single-assignment (no per-step cross-engine
    # WAR semaphores on a persistent tile).
    cur_w = [None, None]
    for dd in range(2):
        w0 = gp.tile([128, 80], f32, tag=f"wk{dd}", name=f"wk{dd}")
        nc.gpsimd.memset(w0[:, 64:80], 0.0)
        cur_w[dd] = w0

    def step(dd, s):
        bi = s // SB
        t = s if dd == 0 else T - 1 - s
        l = t - t0_of(dd, bi)
        zt = ztile[dd][bi]
        ztr = zt[:].rearrange("p (m l b) -> p m l b", l=SB, m=8, b=8)
        rec_first = rec_last = None
        if s > 0:
            tp = t - 1 if dd == 0 else t + 1
            # o-gate chunks (m=0,1) issue LAST so the [i,f,j] sigmoid can
            # start after only 12 of the 16 matmuls — the o sigmoid runs as a
            # separate ACT op off the critical chain (h needs it only after
            # tanh(c), by which time it is long done)
            for m in (2, 3, 4, 5, 6, 7, 0, 1):
                for k in range(2):
                    mm = nc.tensor.matmul(out=ztr[:, m, l, :],
                                          lhsT=whr[dd][:, k * 8 + m, :],
                                          rhs=hr[dd][:, k, tp, :],
                                          start=False, stop=(k == 1))
                    rec_last = mm
                    if rec_first is None:
                        rec_first = mm
        w = cur_w[dd]
        nxt = gp.tile([128, 80], f32, tag=f"wk{dd}", name=f"wk{dd}")
        cur_w[dd] = nxt
        o = {"rec_first": rec_first, "rec_last": rec_last}
        # sigmoid over [i,f,j2] (j-weights doubled so sig_j2 = sigmoid(2j)
        # and tanh(j) = 2*sig_j2 - 1); o-gates in a separate ACT op
        o["sig"] = nc.scalar.activation(
            w[:, 16:64].rearrange("p (m b) -> p m b", b=8),
            ztr[:, 2:8, l, :], AF.Sigmoid)
        o["sigo"] = nc.scalar.activation(
            w[:, 0:16].rearrange("p (m b) -> p m b", b=8),
            ztr[:, 0:2, l, :], AF.Sigmoid)
        # paired product: [sig_i*sig_j2 | sig_f*c] in one op
        pm = gp.tile([128, 32], f32, tag="pm", name="pm")
        o["pm"] = nc.vector.tensor_tensor(pm[:], w[:, 16:48], w[:, 48:80],
                                          op=OP.mult)
        # v = 2*sig_i*sig_j2 - sig_i = sig_i * tanh(j)
        vt = gp.tile([128, 16], f32, tag="vt", name="vt")
        o["vt"] = nc.vector.scalar_tensor_tensor(
            out=vt[:], in0=pm[:, 0:16], scalar=2.0, in1=w[:, 16:32],
            op0=OP.mult, op1=OP.subtract)
        # c = sig_f*c + sig_i*tanh(j), written into the NEXT step's work tile
        o["add"] = nc.vector.tensor_tensor(nxt[:, 64:80], vt[:], pm[:, 16:32],
                                           op=OP.add)
        tct = gp.tile([128, 16], f32, tag="tct", name="tct")
        o["tanh"] = nc.scalar.activation(tct[:], nxt[:, 64:80], AF.Tanh)
        o["h"] = nc.vector.tensor_tensor(
            hr[dd][:, :, t, :],
            w[:, 0:16].rearrange("p (k b) -> p k b", k=2),
            tct[:].rearrange("p (k b) -> p k b", k=2),
            op=OP.mult)
        return o

    for op in prefill_ops(0, 0):
        op()
    for op in prefill_ops(1, 0):
        op()
    from concourse.tile_rust import add_dep_helper

    # loss-projection matmuls: token tile ti has both directions' h final
    # once s >= max(16*ti+15, T-1-16*ti), so all tiles except 0 and NTILE-1
    # stream into the late recurrence steps' PE idle windows (no prefill
    # work remains there); the two edge tiles run after the loop.
    lpool = tc.alloc_tile_pool(name="lps", bufs=1, space="PSUM")
    lg = lpool.tile([128, L2], f32, tag="lg", name="lg")

    def loss_ops(ti):
        ops = []
        for kk in range(4):
            def op_l(ti=ti, kk=kk):
                dd, ch = kk // 2, kk % 2
                return nc.tensor.matmul(
                    out=lg[:, ti * 2 : ti * 2 + 2],
                    lhsT=hst[dd][:, ch * T * 8 + ti * 128 :
                                  ch * T * 8 + (ti + 1) * 128],
                    rhs=w_out[:, kk * 2 : kk * 2 + 2],
                    start=(kk == 0), stop=(kk == 3))
            ops.append(op_l)
        return ops

    lq = []
    mid = list(range(1, NTILE - 1))
    # order middle tiles by completion step so each is ready when popped
    mid.sort(key=lambda ti: max(16 * ti + 15, (T - 1) - 16 * ti))
    for ti in mid:
        lq.extend(loss_ops(ti))
    LS0 = T - 8  # first step at which pinned loss MMs start popping

    queues = [[], []]
    pending = []
    for s in range(T):
        if s % SB == 0:
            bi = s // SB
            for dd in range(2):
                queues[dd] = prefill_ops(dd, bi + 1) if bi + 1 < NBLK else []
        popped_all = []
        so = [None, None]
        for dd in range(2):
            for _ in range(3):
                if queues[dd]:
                    popped_all.append(queues[dd].pop(0)())
            so[dd] = step(dd, s)
        if s >= LS0:
            for _ in range(8):
                if lq:
                    popped_all.append(lq.pop(0)())
        rec_f_first = so[0]["rec_first"]
        rec_b_last = so[1]["rec_last"]
        # pin the steady-state DVE/ACT interleave (fw offset half a period
        # ahead of bw): pmA vtA addA pmB vtB addB hA hB / sigA sigB tanA tanB.
        # Soft deps only — keeps the scheduler from coupling the two serial
        # chains in an order that stretches the step period.
        oa, ob = so
        add_dep_helper(ob["pm"].ins, oa["add"].ins, sync=False,
                       reason="dve interleave")
        add_dep_helper(oa["sigo"].ins, oa["sig"].ins, sync=False,
                       reason="act interleave")
        add_dep_helper(ob["sig"].ins, oa["sigo"].ins, sync=False,
                       reason="act interleave")
        add_dep_helper(ob["sigo"].ins, ob["sig"].ins, sync=False,
                       reason="act interleave")
        add_dep_helper(oa["tanh"].ins, ob["sigo"].ins, sync=False,
                       reason="act interleave")
        add_dep_helper(oa["h"].ins, ob["add"].ins, sync=False,
                       reason="dve interleave")
        add_dep_helper(ob["h"].ins, oa["h"].ins, sync=False,
                       reason="dve interleave")
        # pin prefill into the inter-step PE idle window: after BOTH dirs'
        # recurrent matmuls of this step, before the next step's first
        if rec_f_first is not None:
            for pi in pending:
                add_dep_helper(rec_f_first.ins, pi.ins, sync=False,
                               reason="prefill before next-step rec")
        if rec_b_last is not None:
            for pi in popped_all:
                add_dep_helper(pi.ins, rec_b_last.ins, sync=False,
                               reason="prefill after this-step rec")
            pending = popped_all
        else:
            pending = pending + popped_all
    for dd in range(2):
        for op in queues[dd]:
            op()
    for op in lq:
        op()

    # ---------------- output projection: edge tiles + writeback ------------
    for ti in (0, NTILE - 1) if NTILE > 1 else (0,):
        for op in loss_ops(ti):
            op()
    nc.vector.tensor_copy(lg_sb[:], lg[:])
    nc.sync.dma_start(d["logits"].ap(), lg_sb[:])
    lpool.release()
    gp.release()
    zpool[1].release()
    zpool[0].release()
    pg.release()
    persist.release()


def _build(T=T_FULL):
    if T in _CACHE:
        return _CACHE[T]
    from concourse import bacc, mybir, tile

    f32 = mybir.dt.float32
    bf16 = mybir.dt.bfloat16
    nc = bacc.Bacc("TRN2", target_bir_lowering=False, debug=False,
                   enable_asserts=False, num_devices=NCORE)
    NTOK = BL * T
    NTILE = NTOK // 128
    d = {
        "idx": nc.dram_tensor("idx", [128, NTILE], mybir.dt.int32,
                              kind="ExternalInput"),
        "emb": nc.dram_tensor("emb", [V, E], bf16, kind="ExternalInput"),
        "wx": nc.dram_tensor("wx", [2, 128, 2048], bf16, kind="ExternalInput"),
        "wh": nc.dram_tensor("wh", [2, 128, 2048], bf16, kind="ExternalInput"),
        "bias": nc.dram_tensor("bias", [2, G], bf16, kind="ExternalInput"),
        "wout": nc.dram_tensor("wout", [128, 8], bf16, kind="ExternalInput"),
        "logits": nc.dram_tensor("logits", [128, 2 * NTILE], f32,
                                 kind="ExternalOutput"),
    }
    with tile.TileContext(nc) as tc:
        _emit(nc, tc, d, T)
    nc.compile()
    _CACHE[T] = (nc, d)
    return nc, d


GATE_PERM = np.r_[768:1024, 0:256, 512:768, 256:512]   # [o, i, f, j]


def _stage_core(core, inputs, outputs, mask, emb16, Wk_f, b_f, Wk_b, b_b, W, b, T):
    """Build the per-core input map (pure slicing / transposition / layout).
    emb16 is the embedding table already cast to bf16 (shared across cores)."""
    import ml_dtypes

    k8 = core * BL
    NTOK = BL * T
    NTILE = NTOK // 128
    idx = np.ascontiguousarray(
        inputs[k8 : k8 + BL, :T].T.reshape(NTOK).reshape(NTILE, 128).T
    ).astype(np.int32)
    bf = ml_dtypes.bfloat16
    wx = np.empty((2, 128, 2048), bf)
    wh = np.empty((2, 128, 2048), bf)
    bias = np.empty((2, G), bf)
    for dd, (Wk, bb) in enumerate(((Wk_f, b_f), (Wk_b, b_b))):
        Wp = np.asarray(Wk, np.float32)[:, GATE_PERM].copy()
        bp = np.asarray(bb, np.float32)[GATE_PERM].copy()
        # TF LSTMCell forget bias (permuted order o,i,f,j -> f at 512:768)
        bp[512:768] += 1.0
        # tanh(j) = 2*sigmoid(2j)-1: double the j-gate weights and bias so the
        # one big sigmoid op covers j too (x2 is exact in bf16)
        Wp[:, 768:1024] *= 2.0
        bp[768:1024] *= 2.0
        wx[dd] = (Wp[:E].reshape(2, 128, 8, 128).transpose(1, 0, 2, 3)
                  .reshape(128, 2048).astype(bf))
        wh[dd] = (Wp[E:].reshape(2, 128, 8, 128).transpose(1, 0, 2, 3)
                  .reshape(128, 2048).astype(bf))
        bias[dd] = bp.astype(bf)
    wout = W.reshape(4, 128, 2).transpose(1, 0, 2).reshape(128, 8).astype(bf)
    return {
        "idx": idx,
        "emb": emb16,
        "wx": wx, "wh": wh, "bias": bias,
        "wout": np.ascontiguousarray(wout),
    }


def run(inputs, outputs, mask, emb, Wk_f, b_f, Wk_b, b_b, W, b,
        T=T_FULL, trace=False):
    from concourse import bass_utils

    import ml_dtypes

    nc, d = _build(T)
    emb16 = np.ascontiguousarray(
        np.asarray(emb, np.float32).astype(ml_dtypes.bfloat16))
    args = (np.asarray(inputs), np.asarray(outputs, np.float32),
            np.asarray(mask, np.float32), emb16,
            np.asarray(Wk_f, np.float32), np.asarray(b_f, np.float32),
            np.asarray(Wk_b, np.float32), np.asarray(b_b, np.float32),
            np.asarray(W, np.float32), np.asarray(b, np.float32))
    in_maps = [_stage_core(kc, *args, T) for kc in range(NCORE)]
    res = bass_utils.run_bass_kernel_spmd(nc, in_maps, core_ids=list(range(NCORE)),
                                          trace=trace)
    NTOK = BL * T
    NTILE = NTOK // 128
    # host tail: assemble logits, add b, float64 log-softmax xent, mean
    logits = np.empty((B, T, 2), np.float64)
    for kc in range(NCORE):
        lo = np.asarray(res.results[kc]["logits"], np.float64)   # [128, 2*NTILE]
        lo = lo.reshape(128, NTILE, 2).transpose(1, 0, 2).reshape(NTOK, 2)
        logits[kc * BL : (kc + 1) * BL] = lo.reshape(T, BL, 2).transpose(1, 0, 2)
    logits += np.asarray(b, np.float64)
    m = logits.max(-1, keepdims=True)
    lsm = logits - (m + np.log(np.exp(logits - m).sum(-1, keepdims=True)))
    xent = -(np.asarray(outputs, np.float64)[:, :T] * lsm).sum(-1)
    loss = np.float32((xent * np.asarray(mask, np.float64)[:, :T]).mean())
    return np.asarray(loss), res


def kernel(inputs, outputs, mask, emb, Wk_f, b_f, Wk_b, b_b, W, b):
    loss, _ = run(inputs, outputs, mask, emb, Wk_f, b_f, Wk_b, b_b, W, b)
    return loss


# revision 27
# speedup vs baseline: 1.2537x; 1.0002x over previous
"""Bass/Trainium2 kernel for a BiLSTM (TF-LSTMCell) cross-entropy loss.

Model (per reference):
  x = emb[inputs]                        # [B,T,E]
  h_fw = LSTM(x, Wk_f, b_f)              # forward over T
  h_bw = reverse(LSTM(reverse(x), Wk_b, b_b))
  logits = concat(h_fw, h_bw) @ W + b    # [B,T,2]
  loss = mean(xent(logits, outputs) * mask)

Sharding: data-parallel over batch. B=64 -> 8 cores x 8 rows.  Each core runs
both LSTM directions (two independent chains interleaved to hide latency) and
produces the pre-bias logits for its tokens; the host adds b, computes the
log-softmax cross entropy in float64 and averages (same split as summing the 8
per-core partials: the device does all O(B*T*H) work, the host the O(B*T) tail,
which also avoids an ACT-table swap for LN on device).

Device layout is feature-major: gate/feature index on the SBUF partition axis,
(time x batch) on the free axis, so per-step elementwise work is [128, small]
and the recurrent matmul keeps the weight stationary (bf16 -> fast weight
load).  z pre-activations accumulate in PSUM in 8-step blocks:
  psum col = m*64 + l*8 + b   (l=step-in-block, m=gate chunk of 128, b=batch)
Gate order is host-permuted to [o, i, f, j] so sigmoid covers one contiguous
[128,48] slice and tanh(j) one [128,16].  Weight prep (forget bias +1, j-gate
x2 for the tanh-via-sigmoid trick, bf16 cast, permutation) happens on the host
so weights are DMA-ready.  x-projection and bias are injected into each PSUM
block ahead of time by large-N matmuls (bias via a K=1 ones-row matmul),
keeping the serial chain per step minimal:
  rec-MM (16 bf16 matmuls) -> sigmoid/tanh (ACT) -> cell update (DVE) ->
  tanh(c) (ACT) -> h write (DVE, bf16) -> next rec-MM.

The embedding gather + PE transposes for the input tokens are pipelined into
the first recurrence steps (only the two tiles needed by block 0 are done up
front), so the recurrence starts ~13us into the kernel instead of ~43us.
"""

import numpy as np

B, T_FULL, V, E, H = 64, 256, 32000, 256, 256
G = 4 * H            # 1024 gate dim
NCORE = 8
BL = B // NCORE      # 8 batch rows per core
SB = 8               # recurrence steps per PSUM block

_CACHE = {}


def _emit(nc, tc, d, T):
    """Emit the whole kernel under TileContext tc. d = dict of dram handles."""
    from concourse import bass, mybir

    f32 = mybir.dt.float32
    bf16 = mybir.dt.bfloat16
    AF = mybir.ActivationFunctionType
    OP = mybir.AluOpType

    NTOK = BL * T
    NTILE = NTOK // 128
    NBLK = T // SB
    L2 = 2 * NTILE       # logits tile free dim (classes packed)

    persist = tc.alloc_tile_pool(name="persist", bufs=1)

    # ---------------- persistent SBUF buffers ----------------
    idx_sb = persist.tile([128, NTILE], mybir.dt.int32, tag="idx", name="idx")
    ident = persist.tile([128, 128], bf16, tag="ident", name="ident")
    xT = persist.tile([128, 2 * NTOK], bf16, tag="xT", name="xT")  # [p, k(2), tok]
    wx = [persist.tile([128, 2048], bf16, tag=f"wx{dd}", name=f"wx{dd}") for dd in range(2)]
    wh = [persist.tile([128, 2048], bf16, tag=f"wh{dd}", name=f"wh{dd}") for dd in range(2)]
    bias16 = [persist.tile([1, G], bf16, tag=f"bias16_{dd}", name=f"bias16_{dd}")
              for dd in range(2)]
    hst = [persist.tile([128, 16 * T], bf16, tag=f"h{dd}", name=f"h{dd}") for dd in range(2)]
    ones64 = persist.tile([1, 64], bf16, tag="ones64", name="ones64")
    w_out = persist.tile([128, 8], bf16, tag="w_out", name="w_out")
    lg_sb = persist.tile([128, L2], f32, tag="lg_sb", name="lg_sb")

    # ---------------- constants (gpsimd, ahead of the gathers) -------------
    from concourse.masks import make_identity

    make_identity(nc, ident[:])
    nc.gpsimd.memset(ones64[:], 1.0)

    # ---------------- weight / index loads (sync + scalar queues) ----------
    nc.sync.dma_start(idx_sb[:], d["idx"].ap())
    nc.sync.dma_start(wx[0][:], d["wx"].ap()[0])
    nc.sync.dma_start(wh[0][:], d["wh"].ap()[0])
    nc.scalar.dma_start(wx[1][:], d["wx"].ap()[1])
    nc.scalar.dma_start(wh[1][:], d["wh"].ap()[1])
    for dd in range(2):
        nc.sync.dma_start(bias16[dd][:], d["bias"].ap()[dd : dd + 1])
    nc.scalar.dma_start(w_out[:], d["wout"].ap())

    # ---------------- stage A: gather + xbar transpose ----------------
    # bf16 embedding rows are gathered per 128-token tile (gpsimd SWDGE),
    # then transposed feature-major entirely on the DMA xbar (sync queue) --
    # no PE/ACT/DVE involvement, so stage A never perturbs the recurrence.
    xTr = xT[:].rearrange("p (k n) -> p k n", k=2)
    # interleave from both ends: fw consumes tile 0 first, bw tile NTILE-1
    order = []
    for i in range((NTILE + 1) // 2):
        order.append(i)
        if NTILE - 1 - i > i:
            order.append(NTILE - 1 - i)
    pg = tc.alloc_tile_pool(name="gather", bufs=1)
    xgs = {}

    def gather(i):
        xg = pg.tile([128, E], bf16, tag=f"xg{i}", name=f"xg{i}")
        xgs[i] = xg
        nc.gpsimd.indirect_dma_start(
            out=xg[:], out_offset=None, in_=d["emb"].ap(),
            in_offset=bass.IndirectOffsetOnAxis(ap=idx_sb[:, i : i + 1], axis=0),
        )

    def transpose(i):
        for k in range(2):
            nc.sync.dma_start_transpose(
                out=xTr[:, k, i * 128 : (i + 1) * 128],
                in_=xgs[i][:, k * 128 : (k + 1) * 128])

    # block 0's two tiles gate the first recurrence step: gather them first
    # and transpose on the (idle, cold) PE so they don't queue behind the
    # sem-recycled DMA-transpose stream.  Everything else trails on the
    # gpsimd/sync queues with tens of microseconds of slack.
    for i in order[:2]:
        gather(i)
    with tc.tile_pool(name="tps", bufs=2, space="PSUM") as pps:
        for i in order[:2]:
            for k in range(2):
                ps = pps.tile([128, 128], bf16, tag="tp", name="tp")
                nc.tensor.transpose(out=ps[:], in_=xgs[i][:, k * 128 : (k + 1) * 128],
                                    identity=ident[:])
                nc.vector.tensor_copy(xTr[:, k, i * 128 : (i + 1) * 128], ps[:])
    for i in order[2:]:
        gather(i)
    for i in order[2:]:
        transpose(i)

    # ---------------- recurrence ----------------
    # hst layout: [p, k(2), t(T), b(8)]  (k-major so the loss-stage stationary
    # operand over tokens is a single contiguous free dim)
    hr = [hst[dd][:].rearrange("p (k t b) -> p k t b", k=2, b=8) for dd in range(2)]
    wxr = [wx[dd][:].rearrange("p (q j) -> p q j", j=128) for dd in range(2)]
    whr = [wh[dd][:].rearrange("p (q j) -> p q j", j=128) for dd in range(2)]

    def t0_of(dd, bi):
        return bi * SB if dd == 0 else T - SB - bi * SB

    zpool = [tc.alloc_tile_pool(name=f"z{dd}", bufs=2, space="PSUM")
             for dd in range(2)]
    ztile = [{}, {}]

    def prefill_ops(dd, bi):
        """Closures emitting x-proj + bias matmuls for block bi of dir dd."""
        zt = zpool[dd].tile([128, SB * 64], f32, tag=f"zt{dd}", name=f"zt{dd}")
        ztile[dd][bi] = zt
        # m-major: col = m*64 + l*8 + b -> x-proj/bias matmuls write contiguous
        # column ranges (strided PSUM out-APs measured ~7x slower per matmul)
        ztr = zt[:].rearrange("p (m l b) -> p m l b", l=SB, m=8, b=8)
        t0 = t0_of(dd, bi)
        # local index of global step s=0 in this block (block 0 only);
        # that region gets no recurrent matmul, so bias must close the group
        l_s0 = None
        if bi == 0:
            l_s0 = (0 - t0) if dd == 0 else (T - 1 - t0)
        ops = []
        for m in range(8):
            for k in range(2):
                def op_x(m=m, k=k):
                    return nc.tensor.matmul(
                        out=ztr[:, m, :, :],
                        lhsT=wxr[dd][:, k * 8 + m, :],
                        rhs=xTr[:, k, t0 * 8 : (t0 + SB) * 8],
                        start=(k == 0), stop=False)
                ops.append(op_x)

            def op_b(m=m, l_s0=l_s0):
                if l_s0 is None:
                    return nc.tensor.matmul(out=ztr[:, m, :, :],
                                     lhsT=bias16[dd][:, m * 128 : (m + 1) * 128],
                                     rhs=ones64[:, 0 : SB * 8],
                                     start=False, stop=False)
                else:
                    rest = slice(1, SB) if l_s0 == 0 else slice(0, SB - 1)
                    nc.tensor.matmul(out=ztr[:, m, rest, :],
                                     lhsT=bias16[dd][:, m * 128 : (m + 1) * 128],
                                     rhs=ones64[:, 0 : (SB - 1) * 8],
                                     start=False, stop=False)
                    return nc.tensor.matmul(out=ztr[:, m, l_s0, :],
                                     lhsT=bias16[dd][:, m * 128 : (m + 1) * 128],
                                     rhs=ones64[:, 0:8],
                                     start=False, stop=True)
            ops.append(op_b)
        return ops

    gp = tc.alloc_tile_pool(name="gates", bufs=6)

    # rolling per-step work tiles: cols 0:64 = sigmoid(gates) [o,i,f,j2]
    # written at step s, cols 64:80 = c written by step s-1.  Fresh pool tile
    # per step keeps every write single-assignment (no per-step cross-engine
    # WAR semaphores on a persistent tile).
    cur_w = [None, None]
    for dd in range(2):
        w0 = gp.tile([128, 80], f32, tag=f"wk{dd}", name=f"wk{dd}")
        nc.gpsimd.memset(w0[:, 64:80], 0.0)
        cur_w[dd] = w0

    def step(dd, s):
        bi = s // SB
        t = s if dd == 0 else T - 1 - s
        l = t - t0_of(dd, bi)
        zt = ztile[dd][bi]
        ztr = zt[:].rearrange("p (m l b) -> p m l b", l=SB, m=8, b=8)
        rec_first = rec_last = None
        if s > 0:
            tp = t - 1 if dd == 0 else t + 1
            # o-gate chunks (m=0,1) issue LAST so the [i,f,j] sigmoid can
            # start after only 12 of the 16 matmuls — the o sigmoid runs as a
            # separate ACT op off the critical chain (h needs it only after
            # tanh(c), by which time it is long done)
            for m in (2, 3, 4, 5, 6, 7, 0, 1):
                for k in range(2):
                    mm = nc.tensor.matmul(out=ztr[:, m, l, :],
                                          lhsT=whr[dd][:, k * 8 + m, :],
                                          rhs=hr[dd][:, k, tp, :],
                                          start=False, stop=(k == 1))
                    rec_last = mm
                    if rec_first is None:
                        rec_first = mm
        w = cur_w[dd]
        nxt = gp.tile([128, 80], f32, tag=f"wk{dd}", name=f"wk{dd}")
        cur_w[dd] = nxt
        o = {"rec_first": rec_first, "rec_last": rec_last}
        # sigmoid over [i,f,j2] (j-weights doubled so sig_j2 = sigmoid(2j)
        # and tanh(j) = 2*sig_j2 - 1); o-gates in a separate ACT op
        o["sig"] = nc.scalar.activation(
            w[:, 16:64].rearrange("p (m b) -> p m b", b=8),
            ztr[:, 2:8, l, :], AF.Sigmoid)
        o["sigo"] = nc.scalar.activation(
            w[:, 0:16].rearrange("p (m b) -> p m b", b=8),
            ztr[:, 0:2, l, :], AF.Sigmoid)
        # paired product: [sig_i*sig_j2 | sig_f*c] in one op
        pm = gp.tile([128, 32], f32, tag="pm", name="pm")
        o["pm"] = nc.vector.tensor_tensor(pm[:], w[:, 16:48], w[:, 48:80],
                                          op=OP.mult)
        # v = 2*sig_i*sig_j2 - sig_i = sig_i * tanh(j)
        vt = gp.tile([128, 16], f32, tag="vt", name="vt")
        o["vt"] = nc.vector.scalar_tensor_tensor(
            out=vt[:], in0=pm[:, 0:16], scalar=2.0, in1=w[:, 16:32],
            op0=OP.mult, op1=OP.subtract)
        # c = sig_f*c + sig_i*tanh(j), written into the NEXT step's work tile
        o["add"] = nc.vector.tensor_tensor(nxt[:, 64:80], vt[:], pm[:, 16:32],
                                           op=OP.add)
        tct = gp.tile([128, 16], f32, tag="tct", name="tct")
        o["tanh"] = nc.scalar.activation(tct[:], nxt[:, 64:80], AF.Tanh)
        o["h"] = nc.vector.tensor_tensor(
            hr[dd][:, :, t, :],
            w[:, 0:16].rearrange("p (k b) -> p k b", k=2),
            tct[:].rearrange("p (k b) -> p k b", k=2),
            op=OP.mult)
        return o

    for op in prefill_ops(0, 0):
        op()
    for op in prefill_ops(1, 0):
        op()
    from concourse.tile_rust import add_dep_helper

    # loss-projection matmuls: token tile ti has both directions' h final
    # once s >= max(16*ti+15, T-1-16*ti), so all tiles except 0 and NTILE-1
    # stream into the late recurrence steps' PE idle windows (no prefill
    # work remains there); the two edge tiles run after the loop.
    lpool = tc.alloc_tile_pool(name="lps", bufs=1, space="PSUM")
    lg = lpool.tile([128, L2], f32, tag="lg", name="lg")

    def loss_ops(ti):
        ops = []
        for kk in range(4):
            def op_l(ti=ti, kk=kk):
                dd, ch = kk // 2, kk % 2
                return nc.tensor.matmul(
                    out=lg[:, ti * 2 : ti * 2 + 2],
                    lhsT=hst[dd][:, ch * T * 8 + ti * 128 :
                                  ch * T * 8 + (ti + 1) * 128],
                    rhs=w_out[:, kk * 2 : kk * 2 + 2],
                    start=(kk == 0), stop=(kk == 3))
            ops.append(op_l)
        return ops

    lq = []
    mid = list(range(1, NTILE - 1))
    # order middle tiles by completion step so each is ready when popped
    mid.sort(key=lambda ti: max(16 * ti + 15, (T - 1) - 16 * ti))
    for ti in mid:
        lq.extend(loss_ops(ti))
    LS0 = T - 8  # first step at which pinned loss MMs start popping

    queues = [[], []]
    pending = []
    for s in range(T):
        if s % SB == 0:
            bi = s // SB
            for dd in range(2):
                queues[dd] = prefill_ops(dd, bi + 1) if bi + 1 < NBLK else []
        popped_all = []
        so = [None, None]
        for dd in range(2):
            for _ in range(3):
                if queues[dd]:
                    popped_all.append(queues[dd].pop(0)())
            so[dd] = step(dd, s)
        if s >= LS0:
            for _ in range(8):
                if lq:
                    popped_all.append(lq.pop(0)())
        rec_f_first = so[0]["rec_first"]
        rec_b_last = so[1]["rec_last"]
        # pin the steady-state DVE/ACT interleave (fw offset half a period
        # ahead of bw): pmA vtA addA pmB vtB addB hA hB / sigA sigB tanA tanB.
        # Soft deps only — keeps the scheduler from coupling the two serial
        # chains in an order that stretches the step period.
        oa, ob = so
        add_dep_helper(ob["pm"].ins, oa["add"].ins, sync=False,
                       reason="dve interleave")
        add_dep_helper(oa["sigo"].ins, oa["sig"].ins, sync=False,
                       reason="act interleave")
        add_dep_helper(ob["sig"].ins, oa["sigo"].ins, sync=False,
                       reason="act interleave")
        add_dep_helper(oa["tanh"].ins, ob["sig"].ins, sync=False,
                       reason="act interleave")
        add_dep_helper(ob["sigo"].ins, oa["tanh"].ins, sync=False,
                       reason="act interleave")
        add_dep_helper(oa["h"].ins, ob["add"].ins, sync=False,
                       reason="dve interleave")
        add_dep_helper(ob["h"].ins, oa["h"].ins, sync=False,
                       reason="dve interleave")
        # pin prefill into the inter-step PE idle window: after BOTH dirs'
        # recurrent matmuls of this step, before the next step's first
        if rec_f_first is not None:
            for pi in pending:
                add_dep_helper(rec_f_first.ins, pi.ins, sync=False,
                               reason="prefill before next-step rec")
        if rec_b_last is not None:
            for pi in popped_all:
                add_dep_helper(pi.ins, rec_b_last.ins, sync=False,
                               reason="prefill after this-step rec")
            pending = popped_all
        else:
            pending = pending + popped_all
    for dd in range(2):
        for op in queues[dd]:
            op()
    for op in lq:
        op()

    # ---------------- output projection: edge tiles + writeback ------------
    for ti in (0, NTILE - 1) if NTILE > 1 else (0,):
        for op in loss_ops(ti):
            op()
    nc.vector.tensor_copy(lg_sb[:], lg[:])
    nc.sync.dma_start(d["logits"].ap(), lg_sb[:])
    lpool.release()
    gp.release()
    zpool[1].release()
    zpool[0].release()
    pg.release()
    persist.release()


def _build(T=T_FULL):
    if T in _CACHE:
        return _CACHE[T]
    from concourse import bacc, mybir, tile

    f32 = mybir.dt.float32
    bf16 = mybir.dt.bfloat16
    nc = bacc.Bacc("TRN2", target_bir_lowering=False, debug=False,
                   enable_asserts=False, num_devices=NCORE)
    NTOK = BL * T
    NTILE = NTOK // 128
    d = {
        "idx": nc.dram_tensor("idx", [128, NTILE], mybir.dt.int32,
                              kind="ExternalInput"),
        "emb": nc.dram_tensor("emb", [V, E], bf16, kind="ExternalInput"),
        "wx": nc.dram_tensor("wx", [2, 128, 2048], bf16, kind="ExternalInput"),
        "wh": nc.dram_tensor("wh", [2, 128, 2048], bf16, kind="ExternalInput"),
        "bias": nc.dram_tensor("bias", [2, G], bf16, kind="ExternalInput"),
        "wout": nc.dram_tensor("wout", [128, 8], bf16, kind="ExternalInput"),
        "logits": nc.dram_tensor("logits", [128, 2 * NTILE], f32,
                                 kind="ExternalOutput"),
    }
    with tile.TileContext(nc) as tc:
        _emit(nc, tc, d, T)
    nc.compile()
    _CACHE[T] = (nc, d)
    return nc, d


GATE_PERM = np.r_[768:1024, 0:256, 512:768, 256:512]   # [o, i, f, j]


def _stage_core(core, inputs, outputs, mask, emb16, Wk_f, b_f, Wk_b, b_b, W, b, T):
    """Build the per-core input map (pure slicing / transposition / layout).
    emb16 is the embedding table already cast to bf16 (shared across cores)."""
    import ml_dtypes

    k8 = core * BL
    NTOK = BL * T
    NTILE = NTOK // 128
    idx = np.ascontiguousarray(
        inputs[k8 : k8 + BL, :T].T.reshape(NTOK).reshape(NTILE, 128).T
    ).astype(np.int32)
    bf = ml_dtypes.bfloat16
    wx = np.empty((2, 128, 2048), bf)
    wh = np.empty((2, 128, 2048), bf)
    bias = np.empty((2, G), bf)
    for dd, (Wk, bb) in enumerate(((Wk_f, b_f), (Wk_b, b_b))):
        Wp = np.asarray(Wk, np.float32)[:, GATE_PERM].copy()
        bp = np.asarray(bb, np.float32)[GATE_PERM].copy()
        # TF LSTMCell forget bias (permuted order o,i,f,j -> f at 512:768)
        bp[512:768] += 1.0
        # tanh(j) = 2*sigmoid(2j)-1: double the j-gate weights and bias so the
        # one big sigmoid op covers j too (x2 is exact in bf16)
        Wp[:, 768:1024] *= 2.0
        bp[768:1024] *= 2.0
        wx[dd] = (Wp[:E].reshape(2, 128, 8, 128).transpose(1, 0, 2, 3)
                  .reshape(128, 2048).astype(bf))
        wh[dd] = (Wp[E:].reshape(2, 128, 8, 128).transpose(1, 0, 2, 3)
                  .reshape(128, 2048).astype(bf))
        bias[dd] = bp.astype(bf)
    wout = W.reshape(4, 128, 2).transpose(1, 0, 2).reshape(128, 8).astype(bf)
    return {
        "idx": idx,
        "emb": emb16,
        "wx": wx, "wh": wh, "bias": bias,
        "wout": np.ascontiguousarray(wout),
    }


def run(inputs, outputs, mask, emb, Wk_f, b_f, Wk_b, b_b, W, b,
        T=T_FULL, trace=False):
    from concourse import bass_utils

    import ml_dtypes

    nc, d = _build(T)
    emb16 = np.ascontiguousarray(
        np.asarray(emb, np.float32).astype(ml_dtypes.bfloat16))
    args = (np.asarray(inputs), np.asarray(outputs, np.float32),
            np.asarray(mask, np.float32), emb16,
            np.asarray(Wk_f, np.float32), np.asarray(b_f, np.float32),
            np.asarray(Wk_b, np.float32), np.asarray(b_b, np.float32),
            np.asarray(W, np.float32), np.asarray(b, np.float32))
    in_maps = [_stage_core(kc, *args, T) for kc in range(NCORE)]
    res = bass_utils.run_bass_kernel_spmd(nc, in_maps, core_ids=list(range(NCORE)),
                                          trace=trace)
    NTOK = BL * T
    NTILE = NTOK // 128
    # host tail: assemble logits, add b, float64 log-softmax xent, mean
    logits = np.empty((B, T, 2), np.float64)
    for kc in range(NCORE):
        lo = np.asarray(res.results[kc]["logits"], np.float64)   # [128, 2*NTILE]
        lo = lo.reshape(128, NTILE, 2).transpose(1, 0, 2).reshape(NTOK, 2)
        logits[kc * BL : (kc + 1) * BL] = lo.reshape(T, BL, 2).transpose(1, 0, 2)
    logits += np.asarray(b, np.float64)
    m = logits.max(-1, keepdims=True)
    lsm = logits - (m + np.log(np.exp(logits - m).sum(-1, keepdims=True)))
    xent = -(np.asarray(outputs, np.float64)[:, :T] * lsm).sum(-1)
    loss = np.float32((xent * np.asarray(mask, np.float64)[:, :T]).mean())
    return np.asarray(loss), res


def kernel(inputs, outputs, mask, emb, Wk_f, b_f, Wk_b, b_b, W, b):
    loss, _ = run(inputs, outputs, mask, emb, Wk_f, b_f, Wk_b, b_b, W, b)
    return loss
